# revision 9
# baseline (speedup 1.0000x reference)
"""Trainium2 Bass kernel for the pointer-generator decoder step (fp8 redesign).

Contract: kernel(**inputs) takes the FULL unsharded inputs and returns the
FULL [B, V+OOV] output.

Sharding (8 NeuronCores, one SPMD launch):
  * Front end (LSTM step, attention, context, p_gen, fc1) is data-parallel
    over batch (32 rows/core).
  * z1^T (fp8, 32 KB) is AllGathered so every core holds the full batch.
  * fc2 is tensor-parallel over vocab (6250 cols/core); exp(logits) and
    partial softmax denominators are computed on-chip; final normalization,
    the OOV extension and the copy scatter-add run on the host.

Precision: fp8(e4m3, TRN) for the context einsum (att x64), fc1 (inputs x64,
weights x16) and fc2 (z1 x16, weights x32) with DoubleRow perf mode;
fp32/f32r/bf16 elsewhere.  Host-simulated end-to-end rel err ~2.4e-3.
"""

import os
import sys

for _p in ("/opt/trn_rl_repo",):
    if _p not in sys.path and os.path.isdir(_p):
        sys.path.insert(0, _p)

import ml_dtypes
import numpy as np

import concourse.bass as bass
import concourse.bacc as bacc_mod
import concourse.mybir as mybir
import concourse.tile as tile
from concourse.bass_utils import run_bass_kernel_spmd
from concourse.masks import make_identity

NCORES = 8
B = 256           # batch
BC = B // NCORES  # batch shard per core (32)
I = 256           # input dim
H = 512           # hidden dim
A = 400           # attention dim
AC = 100          # a-chunk (4 chunks of 100 partitions)
V = 50000         # vocab
VC = V // NCORES  # vocab shard per core (6250)
VCP = 6256        # padded to %16 for DoubleRow stride
NT = 512          # vocab tile (one psum bank of fp32)

F32 = mybir.dt.float32
F32R = mybir.dt.float32r
BF16 = mybir.dt.bfloat16
FP8 = mybir.dt.float8e4
AF = mybir.ActivationFunctionType
ALU = mybir.AluOpType
AX = mybir.AxisListType
DR = mybir.MatmulPerfMode.DoubleRow

# scaling scheme (all powers of two, exact)
S_ATT = 64.0      # att, ctx, h carried x64 into fp8
S_FC1W = 16.0     # fc1 weights x16  -> z1 psum x1024
S_Z1 = 16.0       # z1 carried x16 into fp8
S_FC2W = 32.0     # fc2 weights x32  -> logits psum x512
S_PG = 128.0      # p_gen dot products accumulate x128


def _bc(ap, parts):
    """Broadcast a DRAM AP across `parts` partitions (0-stride partition dim)."""
    return bass.AP(tensor=ap.tensor, offset=ap.offset, ap=[[0, parts]] + list(ap.ap))


def _pstride(ap, stride, num):
    """Partition-strided view of a PSUM/SBUF AP (rows 0, stride, 2*stride...)."""
    return bass.AP(tensor=ap.tensor, offset=ap.offset,
                   ap=[[stride, num]] + list(ap.ap)[1:])


def _vocab_tiles():
    out = []
    n0 = 0
    while n0 < VCP:
        out.append((n0, min(NT, VCP - n0)))
        n0 += NT
    return out


def build_nc(with_fc1_bias: bool) -> bass.Bass:
    nc = bacc_mod.Bacc("TRN2", target_bir_lowering=False, num_devices=NCORES)

    # ---- external inputs ----
    x0T = nc.dram_tensor("x0T", [I, BC], BF16, kind="ExternalInput")
    esT = nc.dram_tensor("esT", [H, BC], F32, kind="ExternalInput")
    wihT = nc.dram_tensor("wihT", [I, 12 * 128], BF16, kind="ExternalInput")
    bg = nc.dram_tensor("bg", [12 * 128], F32, kind="ExternalInput")
    whsw = nc.dram_tensor("whsw", [H, 2 * A], F32, kind="ExternalInput")
    smallp = nc.dram_tensor("smallp", [2 * A], F32, kind="ExternalInput")  # attb|v64
    pg1b = nc.dram_tensor("pg1b", [I], BF16, kind="ExternalInput")         # x128
    pg2q = nc.dram_tensor("pg2q", [A], FP8, kind="ExternalInput")          # x2
    pg3q = nc.dram_tensor("pg3q", [H], FP8, kind="ExternalInput")          # x2
    fc1wq = nc.dram_tensor("fc1wq", [128, 8 * 1024], FP8, kind="ExternalInput")
    fc2wq = nc.dram_tensor("fc2wq", [128, 8 * VCP], FP8, kind="ExternalInput")
    enco_q = nc.dram_tensor("enco_q", [AC, BC * 4 * A], FP8, kind="ExternalInput")
    if with_fc1_bias:
        fc1bias = nc.dram_tensor("fc1bias", [1024], F32, kind="ExternalInput")

    # ---- external outputs ----
    p_out = nc.dram_tensor("p_out", [B, VC], BF16, kind="ExternalOutput")
    s_out = nc.dram_tensor("s_out", [128, 2], F32, kind="ExternalOutput")
    attcopy_out = nc.dram_tensor("attcopy_out", [BC, A], F32, kind="ExternalOutput")
    gen_out = nc.dram_tensor("gen_out", [1, BC], F32, kind="ExternalOutput")

    RG = [list(range(NCORES))]

    from contextlib import ExitStack

    with tile.TileContext(nc) as tc, ExitStack() as ctx:
        dram = ctx.enter_context(tc.tile_pool(name="dram", bufs=1, space="DRAM"))
        z1g_c = dram.tile([128, 8 * BC], FP8)                    # z1T local [128,8k,32b]
        z1g_full = dram.tile([NCORES, 128, 8 * BC], FP8, addr_space="Shared")

        const = ctx.enter_context(tc.tile_pool(name="const", bufs=1))
        small = ctx.enter_context(tc.tile_pool(name="small", bufs=4))
        eop = ctx.enter_context(tc.tile_pool(name="eop", bufs=8))
        op_ = ctx.enter_context(tc.tile_pool(name="op", bufs=3))
        psA = ctx.enter_context(tc.tile_pool(name="psA", bufs=3, space="PSUM"))
        psC = ctx.enter_context(tc.tile_pool(name="psC", bufs=3, space="PSUM"))
        psT = ctx.enter_context(tc.tile_pool(name="psT", bufs=1, space="PSUM"))
        psG = ctx.enter_context(tc.tile_pool(name="psG", bufs=1, space="PSUM"))

        ident = const.tile([128, 128], F32)
        make_identity(nc, ident)
        ident_bf = const.tile([128, 128], BF16)
        make_identity(nc, ident_bf)

        # ================= DMA issue plan =================
        # sync (SP HWDGE): LSTM/attention consts, then enco (critical), then
        #   fc1w/fc2w bulk - FIFO order on SP's queue gives enco priority.
        # scalar (Act HWDGE): latency-critical small DMAs (ctx psum gathers,
        #   z1 store, z1T gathered loads) - interleaves with SP's bulk.
        # gpsimd (SWDGE): tiny consts, p_out stores, collective trigger.
        x0T_sb = const.tile([128, 2, BC], BF16)
        nc.sync.dma_start(out=x0T_sb, in_=x0T[:].rearrange("(k p) b -> p k b", p=128))
        esT_sb = const.tile([128, 4, BC], F32R)
        nc.sync.dma_start(out=esT_sb, in_=esT[:].bitcast(F32R).rearrange("(k p) b -> p k b", p=128))
        wih_sb = const.tile([128, 2, 12 * 128], BF16)
        nc.sync.dma_start(out=wih_sb, in_=wihT[:].rearrange("(k p) m -> p k m", p=128))
        whsw_sb = const.tile([128, 4, 2 * A], F32R)
        nc.sync.dma_start(out=whsw_sb, in_=whsw[:].bitcast(F32R).rearrange("(k p) a -> p k a", p=128))

        # enco: 8 groups of 4 batch rows, [100, 4b, 4k, 400] fp8 each
        eo_tiles = []
        for g in range(8):
            eo = eop.tile([AC, 4, 4, A], FP8, tag=f"eo{g}", bufs=1)
            nc.sync.dma_start(
                out=eo,
                in_=enco_q[:, g * 4 * 4 * A:(g + 1) * 4 * 4 * A]
                .rearrange("p (b k e) -> p b k e", b=4, k=4),
            )
            eo_tiles.append(eo)

        fc1w_sb = const.tile([128, 8, 1024], FP8)
        nc.sync.dma_start(out=fc1w_sb, in_=fc1wq[:].rearrange("p (k m) -> p k m", k=8))
        fc2w_sb = const.tile([128, 8, VCP], FP8)
        for t, (n0, nt) in enumerate(_vocab_tiles()):
            nc.sync.dma_start(
                out=fc2w_sb[:, :, n0:n0 + nt],
                in_=fc2wq[:].rearrange("p (k j) -> p k j", k=8)[:, :, n0:n0 + nt],
            )

        # tiny consts on gpsimd
        bg_sb = const.tile([128, 12], F32)
        nc.gpsimd.dma_start(out=bg_sb, in_=bg[:].rearrange("(m p) -> p m", p=128))
        smallc = const.tile([BC, 2 * A], F32)
        nc.gpsimd.dma_start(out=smallc, in_=_bc(smallp[:], BC))
        attb_sb = smallc[:, 0:A]
        v64_sb = smallc[:, A:2 * A]
        pg1_sb = const.tile([128, 2], BF16)
        nc.gpsimd.dma_start(out=pg1_sb, in_=pg1b[:].rearrange("(k p) -> p k", p=128))
        pg2_sb = const.tile([AC, 4], FP8)
        nc.gpsimd.dma_start(out=pg2_sb, in_=pg2q[:].rearrange("(k p) -> p k", p=AC))
        pg3_sb = const.tile([128, 4], FP8)
        nc.gpsimd.dma_start(out=pg3_sb, in_=pg3q[:].rearrange("(k p) -> p k", p=128))
        if with_fc1_bias:
            fc1bias_sb = const.tile([BC, 1024], F32)
            nc.gpsimd.dma_start(out=fc1bias_sb, in_=_bc(fc1bias[:], BC))

        ones_sb = small.tile([1, 1], F32)
        nc.vector.memset(ones_sb, 1.0)
        c64_sb = small.tile([BC, 1], F32)
        nc.vector.memset(c64_sb, 1.0 / S_ATT)

        # ================= LSTM step (h only) =================
        sg_sb = const.tile([128, 12, BC], F32)  # sig(i), tanh(g), sig(o)
        for m in range(12):
            ps_g = psA.tile([128, BC], F32, tag="mmA")
            for k in range(2):
                nc.tensor.matmul(
                    out=ps_g,
                    lhsT=wih_sb[:, k, m * 128:(m + 1) * 128],
                    rhs=x0T_sb[:, k, :],
                    start=(k == 0), stop=(k == 1),
                )
            func = AF.Tanh if 4 <= m < 8 else AF.Sigmoid
            nc.scalar.activation(
                out=sg_sb[:, m, :], in_=ps_g, func=func,
                bias=bg_sb[:, m:m + 1], scale=1.0,
            )
        cth_sb = const.tile([128, 4, BC], F32)
        nc.vector.tensor_mul(out=cth_sb, in0=sg_sb[:, 0:4, :], in1=sg_sb[:, 4:8, :])
        nc.scalar.activation(out=cth_sb, in_=cth_sb, func=AF.Tanh)
        hT_sb = const.tile([128, 4, BC], F32R)  # h feature-major (attention lhsT)
        nc.vector.tensor_mul(out=hT_sb, in0=sg_sb[:, 8:12, :], in1=cth_sb)

        # dec_inT fp8 [128, 8, 32]: slots 0-3 ctx x64 (100 partitions), 4-7 h x64
        dec_inT = const.tile([128, 8, BC], FP8)
        nc.scalar.activation(out=dec_inT[:, 4:8, :], in_=hT_sb.bitcast(F32),
                             func=AF.Copy, scale=S_ATT)

        # ================= attention =================
        ps_e = psA.tile([BC, A], F32, tag="mmA")
        for k in range(4):
            nc.tensor.matmul(out=ps_e, lhsT=esT_sb[:, k, :], rhs=whsw_sb[:, k, 0:A],
                             start=(k == 0), stop=False)
        for k in range(4):
            nc.tensor.matmul(out=ps_e, lhsT=hT_sb[:, k, :], rhs=whsw_sb[:, k, A:2 * A],
                             start=False, stop=(k == 3))
        e_sb = const.tile([BC, A], F32)
        nc.vector.scalar_tensor_tensor(out=e_sb, in0=ps_e, scalar=1.0, in1=attb_sb,
                                       op0=ALU.mult, op1=ALU.add)
        nc.scalar.activation(out=e_sb, in_=e_sb, func=AF.Tanh)
        mneg = small.tile([BC, 1], F32)
        nc.vector.tensor_reduce(out=mneg, in_=e_sb, axis=AX.X, op=ALU.max, negate=True)
        ssum = small.tile([BC, 1], F32)
        nc.scalar.activation(out=e_sb, in_=e_sb, func=AF.Exp, bias=mneg, scale=1.0,
                             accum_out=ssum)
        rs = small.tile([BC, 1], F32)
        nc.vector.reciprocal(out=rs, in_=ssum)
        att64_sb = const.tile([BC, A], F32)  # att x 64
        nc.vector.scalar_tensor_tensor(out=att64_sb, in0=e_sb, scalar=rs, in1=v64_sb,
                                       op0=ALU.mult, op1=ALU.mult)

        # attT fp8 [100p, 4, 32] via PE transposes (chunks of 100)
        attT_q = const.tile([AC, 4, BC], FP8)
        for c in range(4):
            ps_t = psT.tile([128, BC], F32, tag="tp")
            nc.tensor.transpose(ps_t[:AC, :], att64_sb[:, c * AC:(c + 1) * AC],
                                ident[:BC, :BC])
            nc.scalar.copy(out=attT_q[:, c, :], in_=ps_t[:AC, :])

        # ================= context: ctx64[b,:] = att64[b] @ enco[b] =================
        # Per-row [1,400] psum results; psum->SBUF copies (partition 0 scratch,
        # alternating Scalar/Vector) hide under the enco DMA; 4 chunked
        # SBUF->SBUF DMAs restore batch-major layout.
        ctx_sb = const.tile([BC, A], F32)  # ctx x 64, batch-major
        rows_sb = const.tile([1, BC, A], F32)
        for b in range(BC):
            pst = psC.tile([1, A], F32, tag="psc")
            eo = eo_tiles[b // 4]
            for p in range(2):
                nc.tensor.matmul(
                    out=pst,
                    lhsT=attT_q[:, 2 * p:2 * p + 2, b:b + 1],
                    rhs=eo[:, b % 4, 2 * p:2 * p + 2, :],
                    start=(p == 0), stop=(p == 1),
                    perf_mode=DR,
                )
            if b % 2 == 0:
                nc.scalar.copy(out=rows_sb[:, b, :], in_=pst)
            else:
                nc.vector.tensor_copy(out=rows_sb[:, b, :], in_=pst)
            if b % 8 == 7:
                nc.scalar.dma_start(out=ctx_sb[b - 7:b + 1, :],
                                    in_=rows_sb[:, b - 7:b + 1, :])

        # ctxT fp8 -> dec_inT slots 0-3 (x64 already)
        for c in range(4):
            ps_t = psT.tile([128, BC], F32, tag="tp")
            nc.tensor.transpose(ps_t[:AC, :], ctx_sb[:, c * AC:(c + 1) * AC],
                                ident[:BC, :BC])
            nc.scalar.copy(out=dec_inT[:AC, c, :], in_=ps_t[:AC, :])

        # ================= fc1: z1 = dec_in @ fc1_w^T (+bias) =================
        z1_sb = const.tile([BC, 1024], BF16)
        for nh in range(2):
            ps_z = psA.tile([BC, NT], F32, tag="mmA")
            ns = slice(nh * NT, (nh + 1) * NT)
            for p in range(2):  # ctx pairs, K=100
                nc.tensor.matmul(
                    out=ps_z,
                    lhsT=dec_inT[:AC, 2 * p:2 * p + 2, :],
                    rhs=fc1w_sb[:AC, 2 * p:2 * p + 2, ns],
                    start=(p == 0), stop=False, perf_mode=DR,
                )
            for p in range(2):  # h pairs, K=128
                nc.tensor.matmul(
                    out=ps_z,
                    lhsT=dec_inT[:, 4 + 2 * p:4 + 2 * p + 2, :],
                    rhs=fc1w_sb[:, 4 + 2 * p:4 + 2 * p + 2, ns],
                    start=False, stop=(p == 1), perf_mode=DR,
                )
            if with_fc1_bias:
                nc.vector.scalar_tensor_tensor(
                    out=z1_sb[:, ns], in0=ps_z, scalar=1.0 / (S_ATT * S_FC1W),
                    in1=fc1bias_sb[:, ns], op0=ALU.mult, op1=ALU.add,
                )
            else:
                nc.scalar.activation(out=z1_sb[:, ns], in_=ps_z, func=AF.Copy,
                                     scale=1.0 / (S_ATT * S_FC1W))

        # z1T fp8 (x16) [128, 8, 32] -> DRAM -> AllGather
        z1loc = const.tile([128, 8, BC], FP8)
        for j in range(8):
            ps_t = psT.tile([128, BC], BF16, tag="tp")
            nc.tensor.transpose(ps_t, z1_sb[:, j * 128:(j + 1) * 128],
                                ident_bf[:BC, :BC])
            nc.scalar.activation(out=z1loc[:, j, :], in_=ps_t, func=AF.Copy,
                                 scale=S_Z1)
        nc.scalar.dma_start(out=z1g_c[:], in_=z1loc)
        nc.gpsimd.collective_compute(
            "AllGather", ALU.bypass, replica_groups=RG,
            ins=[z1g_c.opt()], outs=[z1g_full.opt()],
        )

        # ================= p_gen (all dots on PE, accumulate x128) =================
        ps_gen = psG.tile([1, BC], F32, tag="gen")
        for k in range(2):  # x0 . pg1  (pg1 x128, bf16)
            nc.tensor.matmul(out=ps_gen, lhsT=pg1_sb[:, k:k + 1], rhs=x0T_sb[:, k, :],
                             start=(k == 0), stop=False)
        for c in range(4):  # ctx . pg2 (ctxT x64 * pg2 x2, fp8)
            nc.tensor.matmul(out=ps_gen, lhsT=pg2_sb[:, c:c + 1],
                             rhs=dec_inT[:AC, c, :], start=False, stop=False)
        for c in range(4):  # h . pg3 (hT x64 * pg3 x2, fp8)
            nc.tensor.matmul(out=ps_gen, lhsT=pg3_sb[:, c:c + 1],
                             rhs=dec_inT[:, 4 + c, :], start=False, stop=(c == 3))
        gen_row = small.tile([1, BC], F32)
        nc.scalar.activation(out=gen_row, in_=ps_gen, func=AF.Sigmoid,
                             scale=1.0 / S_PG)
        nc.scalar.dma_start(out=gen_out[:], in_=gen_row)
        # gen column [32,1] via 1-k matmul with ones
        ps_gc = psT.tile([BC, 1], F32, tag="tp")
        nc.tensor.matmul(out=ps_gc, lhsT=gen_row, rhs=ones_sb)
        gen1m64 = small.tile([BC, 1], F32)
        nc.scalar.activation(out=gen1m64, in_=ps_gc, func=AF.Identity,
                             bias=c64_sb, scale=-1.0 / S_ATT)
        attcopy_sb = const.tile([BC, A], F32)
        nc.vector.tensor_scalar_mul(out=attcopy_sb, in0=att64_sb, scalar1=gen1m64)
        nc.scalar.dma_start(out=attcopy_out[:], in_=attcopy_sb)

        # ================= gathered z1T [128, 8, 256] =================
        z1T_all = const.tile([128, 8, B], FP8)
        for c in range(NCORES):
            nc.scalar.dma_start(
                out=z1T_all[:, :, c * BC:(c + 1) * BC],
                in_=z1g_full[c].rearrange("p (k b) -> p k b", k=8),
            )

        # ================= fc2 + exp + partial denominators =================
        s_acc = small.tile([128, 2], F32, tag="sacc")
        nc.vector.memset(s_acc, 0.0)
        for n0, nt in _vocab_tiles():
            nreal = min(nt, VC - n0)
            for bh in range(2):
                ps_l = psA.tile([128, NT], F32, tag="mmA")
                for p in range(4):
                    nc.tensor.matmul(
                        out=ps_l[:, :nt],
                        lhsT=z1T_all[:, 2 * p:2 * p + 2, bh * 128:(bh + 1) * 128],
                        rhs=fc2w_sb[:, 2 * p:2 * p + 2, n0:n0 + nt],
                        start=(p == 0), stop=(p == 3), perf_mode=DR,
                    )
                o_sb = op_.tile([128, NT], BF16, tag="osb")
                st = small.tile([128, 1], F32, tag="st")
                nc.scalar.activation(
                    out=o_sb[:, :nreal], in_=ps_l[:, :nreal], func=AF.Exp,
                    scale=1.0 / (S_Z1 * S_FC2W), accum_out=st,
                )
                nc.vector.tensor_add(out=s_acc[:, bh:bh + 1],
                                     in0=s_acc[:, bh:bh + 1], in1=st)
                nc.gpsimd.dma_start(
                    out=p_out[bh * 128:(bh + 1) * 128, n0:n0 + nreal],
                    in_=o_sb[:, :nreal],
                )
        nc.scalar.dma_start(out=s_out[:], in_=s_acc)

    nc.compile()
    return nc


_NC_CACHE = {}


def _get_nc(with_fc1_bias: bool) -> bass.Bass:
    if with_fc1_bias not in _NC_CACHE:
        _NC_CACHE[with_fc1_bias] = build_nc(with_fc1_bias)
    return _NC_CACHE[with_fc1_bias]


RUN_KW = {}        # test.py can set e.g. {"trace": True}
LAST_RESULT = {}   # test.py reads exec_time_ns etc.


def make_in_maps(inputs: dict):
    f32 = lambda a: np.ascontiguousarray(np.asarray(a), dtype=np.float32)
    bf16 = ml_dtypes.bfloat16
    fp8 = ml_dtypes.float8_e4m3  # TRN variant (max +-240)

    def q8(a):
        return np.clip(a, -240.0, 240.0).astype(fp8)

    x = f32(inputs["x"])[:, 0, :]              # [B, I]
    enco = f32(inputs["encoder_outputs"])      # [B, A, A]
    es = f32(inputs["encoder_state"])          # [B, H]
    W_ih = f32(inputs["W_ih"])                 # [4H, I]
    b = f32(inputs["b_ih"]) + f32(inputs["b_hh"])
    Wh_w = f32(inputs["Wh_w"])
    Ws_w = f32(inputs["Ws_w"])
    attb = f32(inputs["Wh_b"]) + f32(inputs["Ws_b"])
    vvec = f32(inputs["v"])
    fc1_w = f32(inputs["fc1_w"])               # [2H, H+A]
    fc1_b = f32(inputs["fc1_b"])
    fc2_w = f32(inputs["fc2_w"])               # [V, 2H]
    pg1 = f32(inputs["pg1_w"])[0]
    pg2 = f32(inputs["pg2_w"])[0]
    pg3 = f32(inputs["pg3_w"])[0]

    with_fc1_bias = bool(np.any(fc1_b != 0.0))

    # i, g, o gate rows (f gate dead: c0 = 0)
    idx = np.r_[0:H, 2 * H:3 * H, 3 * H:4 * H]
    wihT = np.ascontiguousarray(W_ih[idx].T).astype(bf16)   # [I, 1536]
    bgv = np.ascontiguousarray(b[idx])

    whsw = np.concatenate([Wh_w.T, Ws_w.T], axis=1)          # [H, 2A]
    smallp = np.concatenate([attb, vvec * S_ATT])            # [800]
    pg1b = (pg1 * S_PG).astype(bf16)
    pg2qv = q8(pg2 * 2.0)
    pg3qv = q8(pg3 * 2.0)

    # fc1 weights, prearranged in SBUF layout [128, 8, 1024] (x16)
    fc1T = fc1_w.T * S_FC1W                                  # [912, 1024]
    fc1wq = np.zeros((128, 8, 1024), np.float32)
    for k in range(4):
        fc1wq[:AC, k, :] = fc1T[k * AC:(k + 1) * AC]         # ctx chunks of 100
    for k in range(4):
        fc1wq[:, 4 + k, :] = fc1T[A + k * 128:A + (k + 1) * 128]
    fc1wq = q8(fc1wq.reshape(128, 8 * 1024))

    # fc2 weights [128, 8, 6256] (x32), cols >= 6250 zero
    fc2T = fc2_w.T * S_FC2W                                  # [1024, V]
    fc2wq_f = np.zeros((128, 8, VCP), np.float32)

    enco_pk = np.empty((AC, BC * 4 * A), ml_dtypes.float8_e4m3)

    x0T = np.ascontiguousarray(x.T)
    esTm = np.ascontiguousarray(es.T)

    in_maps = []
    enco_q8 = q8(enco)                                       # [B, A, A] fp8
    for c in range(NCORES):
        bs = slice(c * BC, (c + 1) * BC)
        vs = slice(c * VC, (c + 1) * VC)
        fc2wq_f[:, :, :] = 0.0
        for k in range(8):
            fc2wq_f[:, k, :VC] = fc2T[k * 128:(k + 1) * 128, vs]
        fc2wq = q8(fc2wq_f.reshape(128, 8 * VCP))
        # enco chunk-major: [p, (b, k, e)] = enco[bs][b, k*100+p, e]
        ec = enco_q8[bs]                                     # [32, 400, 400]
        epk = ec.reshape(BC, 4, AC, A).transpose(2, 0, 1, 3)  # [100, 32, 4, 400]
        enco_pk = np.ascontiguousarray(epk.reshape(AC, BC * 4 * A))
        m = {
            "x0T": np.ascontiguousarray(x0T[:, bs]).astype(bf16),
            "esT": np.ascontiguousarray(esTm[:, bs]),
            "wihT": wihT,
            "bg": bgv,
            "whsw": whsw,
            "smallp": smallp,
            "pg1b": pg1b,
            "pg2q": pg2qv,
            "pg3q": pg3qv,
            "fc1wq": fc1wq,
            "fc2wq": fc2wq,
            "enco_q": enco_pk,
        }
        if with_fc1_bias:
            m["fc1bias"] = fc1_b
        in_maps.append(m)
    return in_maps, with_fc1_bias


def kernel(**inputs) -> np.ndarray:
    in_maps, with_fc1_bias = make_in_maps(inputs)
    nc = _get_nc(with_fc1_bias)

    res = run_bass_kernel_spmd(nc, in_maps, core_ids=list(range(NCORES)), **RUN_KW)
    results = res.results
    LAST_RESULT["exec_time_ns"] = getattr(res, "exec_time_ns", None)

    oov = int(np.asarray(inputs["max_oov_nums"]))
    ids = np.asarray(inputs["ids"])
    fc2_b = np.asarray(inputs["fc2_b"], dtype=np.float32)

    gen = np.concatenate(
        [np.asarray(results[c]["gen_out"])[0] for c in range(NCORES)])
    p = np.zeros((B, V + oov), dtype=np.float32)
    if np.any(fc2_b != 0.0):
        g = np.exp(fc2_b.astype(np.float64)).astype(np.float32)
        for c in range(NCORES):
            vs = slice(c * VC, (c + 1) * VC)
            p[:, vs] = np.asarray(results[c]["p_out"], dtype=np.float32) * g[None, vs]
        s = p[:, :V].sum(axis=1)
        p[:, :V] *= (gen / s)[:, None]
    else:
        s = np.zeros(B, dtype=np.float32)
        for c in range(NCORES):
            so = np.asarray(results[c]["s_out"])       # [128, 2]
            s += so.T.reshape(B)
        f = (gen / s).astype(np.float32)
        for c in range(NCORES):
            vs = slice(c * VC, (c + 1) * VC)
            p[:, vs] = np.asarray(results[c]["p_out"], dtype=np.float32) * f[:, None]

    att_copy = np.concatenate(
        [np.asarray(results[c]["attcopy_out"]) for c in range(NCORES)], axis=0)
    rows = np.arange(B)[:, None]
    np.add.at(p, (rows, ids), att_copy)
    return p


# revision 20
# speedup vs baseline: 1.1937x; 1.1937x over previous
"""Trainium2 Bass kernel for the pointer-generator decoder step (fp8 redesign).

Contract: kernel(**inputs) takes the FULL unsharded inputs and returns the
FULL [B, V+OOV] output.

Sharding (8 NeuronCores, one SPMD launch):
  * Front end (LSTM step, attention, context, p_gen, fc1) is data-parallel
    over batch (32 rows/core).
  * z1^T (fp8, 32 KB) is AllGathered so every core holds the full batch.
  * fc2 is tensor-parallel over vocab (6250 cols/core); exp(logits) and
    partial softmax denominators are computed on-chip; final normalization,
    the OOV extension and the copy scatter-add run on the host.

Precision: fp8(e4m3, TRN) for the context einsum (att x64), fc1 (inputs x64,
weights x16) and fc2 (z1 x16, weights x32) with DoubleRow perf mode;
fp32/f32r/bf16 elsewhere.  Host-simulated end-to-end rel err ~2.4e-3.
"""

import os
import sys

for _p in ("/opt/trn_rl_repo",):
    if _p not in sys.path and os.path.isdir(_p):
        sys.path.insert(0, _p)

import ml_dtypes
import numpy as np

import concourse.bass as bass
import concourse.bacc as bacc_mod
import concourse.mybir as mybir
import concourse.tile as tile
from concourse.bass_utils import run_bass_kernel_spmd
from concourse.masks import make_identity

NCORES = 8
B = 256           # batch
BC = B // NCORES  # batch shard per core (32)
I = 256           # input dim
H = 512           # hidden dim
A = 400           # attention dim
AC = 100          # a-chunk (4 chunks of 100 partitions)
V = 50000         # vocab
VC = V // NCORES  # vocab shard per core (6250)
VCP = 6256        # padded to %16 for DoubleRow stride
NT = 512          # vocab tile (one psum bank of fp32)

F32 = mybir.dt.float32
F32R = mybir.dt.float32r
BF16 = mybir.dt.bfloat16
FP8 = mybir.dt.float8e4
AF = mybir.ActivationFunctionType
ALU = mybir.AluOpType
AX = mybir.AxisListType
DR = mybir.MatmulPerfMode.DoubleRow

# scaling scheme (all powers of two, exact)
S_ATT = 64.0      # att, ctx, h carried x64 into fp8
S_FC1W = 16.0     # fc1 weights x16  -> z1 psum x1024
S_Z1 = 16.0       # z1 carried x16 into fp8
S_FC2W = 32.0     # fc2 weights x32  -> logits psum x512
S_PG = 128.0      # p_gen dot products accumulate x128


def _bc(ap, parts):
    """Broadcast a DRAM AP across `parts` partitions (0-stride partition dim)."""
    return bass.AP(tensor=ap.tensor, offset=ap.offset, ap=[[0, parts]] + list(ap.ap))


def _pstride(ap, stride, num):
    """Partition-strided view of a PSUM/SBUF AP (rows 0, stride, 2*stride...)."""
    return bass.AP(tensor=ap.tensor, offset=ap.offset,
                   ap=[[stride, num]] + list(ap.ap)[1:])


def _vocab_tiles():
    out = []
    n0 = 0
    while n0 < VCP:
        out.append((n0, min(NT, VCP - n0)))
        n0 += NT
    return out


def build_nc(with_fc1_bias: bool) -> bass.Bass:
    nc = bacc_mod.Bacc("TRN2", target_bir_lowering=False, num_devices=NCORES)

    # ---- external inputs ----
    x0T = nc.dram_tensor("x0T", [I, BC], BF16, kind="ExternalInput")
    esT = nc.dram_tensor("esT", [H, BC], BF16, kind="ExternalInput")
    wihT = nc.dram_tensor("wihT", [I, 12 * 128], BF16, kind="ExternalInput")
    bg = nc.dram_tensor("bg", [12 * 128], F32, kind="ExternalInput")
    whsw = nc.dram_tensor("whsw", [H, 2 * A], BF16, kind="ExternalInput")
    smallp = nc.dram_tensor("smallp", [2 * A], F32, kind="ExternalInput")  # attb|v64
    attb_bf = nc.dram_tensor("attb_bf", [A], BF16, kind="ExternalInput")
    pg1b = nc.dram_tensor("pg1b", [I], BF16, kind="ExternalInput")         # x128
    pg2q = nc.dram_tensor("pg2q", [A], FP8, kind="ExternalInput")          # x2
    pg3q = nc.dram_tensor("pg3q", [H], FP8, kind="ExternalInput")          # x2
    fc1wq = nc.dram_tensor("fc1wq", [128, 8 * 1024], FP8, kind="ExternalInput")
    fc2wq = nc.dram_tensor("fc2wq", [128, 8 * VCP], FP8, kind="ExternalInput")
    enco_q = nc.dram_tensor("enco_q", [AC, BC * 4 * A], FP8, kind="ExternalInput")
    if with_fc1_bias:
        fc1bias = nc.dram_tensor("fc1bias", [1024], F32, kind="ExternalInput")

    # ---- external outputs ----
    p_out = nc.dram_tensor("p_out", [B, VC], BF16, kind="ExternalOutput")
    s_out = nc.dram_tensor("s_out", [128, 2], F32, kind="ExternalOutput")
    attcopy_out = nc.dram_tensor("attcopy_out", [BC, A], F32, kind="ExternalOutput")
    gen_out = nc.dram_tensor("gen_out", [1, BC], F32, kind="ExternalOutput")

    RG = [list(range(NCORES))]

    from contextlib import ExitStack

    with tile.TileContext(nc) as tc, ExitStack() as ctx:
        dram = ctx.enter_context(tc.tile_pool(name="dram", bufs=1, space="DRAM"))
        z1g_c = dram.tile([128, 8 * BC], FP8)                    # z1T local [128,8k,32b]
        z1g_full = dram.tile([NCORES, 128, 8 * BC], FP8, addr_space="Shared")

        const = ctx.enter_context(tc.tile_pool(name="const", bufs=1))
        small = ctx.enter_context(tc.tile_pool(name="small", bufs=4))
        eop = ctx.enter_context(tc.tile_pool(name="eop", bufs=8))
        op_ = ctx.enter_context(tc.tile_pool(name="op", bufs=3))
        psA = ctx.enter_context(tc.tile_pool(name="psA", bufs=3, space="PSUM"))
        psC = ctx.enter_context(tc.tile_pool(name="psC", bufs=3, space="PSUM"))
        psT = ctx.enter_context(tc.tile_pool(name="psT", bufs=1, space="PSUM"))
        psG = ctx.enter_context(tc.tile_pool(name="psG", bufs=1, space="PSUM"))

        ident = const.tile([128, 128], F32)
        make_identity(nc, ident)
        ident_bf = const.tile([128, 128], BF16)
        make_identity(nc, ident_bf)

        # ================= DMA issue plan =================
        # sync (SP HWDGE): LSTM/attention consts, then enco (critical), then
        #   fc1w/fc2w bulk - FIFO order on SP's queue gives enco priority.
        # scalar (Act HWDGE): latency-critical small DMAs (ctx psum gathers,
        #   z1 store, z1T gathered loads) - interleaves with SP's bulk.
        # gpsimd (SWDGE): tiny consts, p_out stores, collective trigger.
        x0T_sb = const.tile([128, 2, BC], BF16)
        nc.sync.dma_start(out=x0T_sb, in_=x0T[:].rearrange("(k p) b -> p k b", p=128))
        wih_sb = const.tile([128, 2, 12 * 128], BF16)
        nc.sync.dma_start(out=wih_sb, in_=wihT[:].rearrange("(k p) m -> p k m", p=128))
        esT_sb = const.tile([128, 4, BC], BF16)
        nc.sync.dma_start(out=esT_sb, in_=esT[:].rearrange("(k p) b -> p k b", p=128))
        whsw_sb = const.tile([128, 4, 2 * A], BF16)
        nc.sync.dma_start(out=whsw_sb, in_=whsw[:].rearrange("(k p) a -> p k a", p=128))

        # fc1w on the Act queue: interleaves with SP's enco stream, in early
        fc1w_sb = const.tile([128, 8, 1024], FP8)
        nc.scalar.dma_start(out=fc1w_sb, in_=fc1wq[:].rearrange("p (k m) -> p k m", k=8))

        # enco: 8 groups of 4 batch rows, [100, 4b, 4k, 400] fp8 each
        eo_tiles = []
        for g in range(8):
            eo = eop.tile([AC, 4, 4, A], FP8, tag=f"eo{g}", bufs=1)
            nc.sync.dma_start(
                out=eo,
                in_=enco_q[:, g * 4 * 4 * A:(g + 1) * 4 * 4 * A]
                .rearrange("p (b k e) -> p b k e", b=4, k=4),
            )
            eo_tiles.append(eo)

        # tiny consts on gpsimd
        bg_sb = const.tile([128, 12], F32)
        nc.gpsimd.dma_start(out=bg_sb, in_=bg[:].rearrange("(m p) -> p m", p=128))
        smallc = const.tile([BC, 2 * A], F32)
        nc.gpsimd.dma_start(out=smallc, in_=_bc(smallp[:], BC))
        attb_sb = smallc[:, 0:A]
        v64_sb = smallc[:, A:2 * A]
        pg1_sb = const.tile([128, 2], BF16)
        nc.gpsimd.dma_start(out=pg1_sb, in_=pg1b[:].rearrange("(k p) -> p k", p=128))
        pg2_sb = const.tile([AC, 4], FP8)
        nc.gpsimd.dma_start(out=pg2_sb, in_=pg2q[:].rearrange("(k p) -> p k", p=AC))
        pg3_sb = const.tile([128, 4], FP8)
        nc.gpsimd.dma_start(out=pg3_sb, in_=pg3q[:].rearrange("(k p) -> p k", p=128))
        if with_fc1_bias:
            fc1bias_sb = const.tile([BC, 1024], F32)
            nc.gpsimd.dma_start(out=fc1bias_sb, in_=_bc(fc1bias[:], BC))

        attbr_sb = const.tile([1, A], BF16)
        nc.gpsimd.dma_start(out=attbr_sb, in_=attb_bf[:].rearrange("(p a) -> p a", p=1))
        ones_sb = small.tile([1, 1], F32)
        nc.vector.memset(ones_sb, 1.0)
        onesb_sb = small.tile([1, BC], BF16)
        nc.vector.memset(onesb_sb, 1.0)
        c64_sb = small.tile([BC, 1], F32)
        nc.vector.memset(c64_sb, 1.0 / S_ATT)

        # ================= LSTM step (h only) =================
        sg_sb = const.tile([128, 12, BC], F32)  # sig(i), tanh(g), sig(o)
        for m in range(12):
            ps_g = psA.tile([128, BC], F32, tag="mmA")
            for k in range(2):
                nc.tensor.matmul(
                    out=ps_g,
                    lhsT=wih_sb[:, k, m * 128:(m + 1) * 128],
                    rhs=x0T_sb[:, k, :],
                    start=(k == 0), stop=(k == 1),
                )
            func = AF.Tanh if 4 <= m < 8 else AF.Sigmoid
            nc.scalar.activation(
                out=sg_sb[:, m, :], in_=ps_g, func=func,
                bias=bg_sb[:, m:m + 1], scale=1.0,
            )
        cth_sb = const.tile([128, 4, BC], F32)
        nc.vector.tensor_mul(out=cth_sb, in0=sg_sb[:, 0:4, :], in1=sg_sb[:, 4:8, :])
        nc.scalar.activation(out=cth_sb, in_=cth_sb, func=AF.Tanh)
        hT_sb = const.tile([128, 4, BC], BF16)  # h feature-major (attention lhsT)
        nc.vector.tensor_mul(out=hT_sb, in0=sg_sb[:, 8:12, :], in1=cth_sb)

        # dec_inT fp8 [128, 8, 32]: slots 0-3 ctx x64 (100 partitions), 4-7 h x64
        dec_inT = const.tile([128, 8, BC], FP8)
        nc.scalar.activation(out=dec_inT[:, 4:8, :], in_=hT_sb,
                             func=AF.Copy, scale=S_ATT)

        # ================= attention =================
        # e psum accumulates es@Wh + h@Ws + attb (attb via a K=1 ones matmul);
        # tanh bounds e, so softmax needs no max subtraction.
        ps_e = psA.tile([BC, A], F32, tag="mmA")
        for k in range(4):
            nc.tensor.matmul(out=ps_e, lhsT=esT_sb[:, k, :], rhs=whsw_sb[:, k, 0:A],
                             start=(k == 0), stop=False)
        for k in range(4):
            nc.tensor.matmul(out=ps_e, lhsT=hT_sb[:, k, :], rhs=whsw_sb[:, k, A:2 * A],
                             start=False, stop=False)
        nc.tensor.matmul(out=ps_e, lhsT=onesb_sb, rhs=attbr_sb,
                         start=False, stop=True)
        e_sb = const.tile([BC, A], F32)
        nc.scalar.activation(out=e_sb, in_=ps_e, func=AF.Tanh)
        ssum = small.tile([BC, 1], F32)
        nc.scalar.activation(out=e_sb, in_=e_sb, func=AF.Exp, accum_out=ssum)
        rs = small.tile([BC, 1], F32)
        nc.vector.reciprocal(out=rs, in_=ssum)
        att64_sb = const.tile([BC, A], F32)  # att x 64
        nc.vector.scalar_tensor_tensor(out=att64_sb, in0=e_sb, scalar=rs, in1=v64_sb,
                                       op0=ALU.mult, op1=ALU.mult)

        # attT fp8 [100p, 4, 32] via PE transposes (chunks of 100)
        attT_q = const.tile([AC, 4, BC], FP8)
        for c in range(4):
            ps_t = psT.tile([128, BC], F32, tag="tp")
            nc.tensor.transpose(ps_t[:AC, :], att64_sb[:, c * AC:(c + 1) * AC],
                                ident[:BC, :BC])
            nc.scalar.copy(out=attT_q[:, c, :], in_=ps_t[:AC, :])

        # ================= context: ctx64[b,:] = att64[b] @ enco[b] =================
        # Per-row [1,400] psum results; psum->SBUF copies (partition 0 scratch,
        # alternating Scalar/Vector) hide under the enco DMA; 4 chunked
        # SBUF->SBUF DMAs restore batch-major layout.
        ctx_sb = const.tile([BC, A], F32)  # ctx x 64, batch-major
        rows_sb = const.tile([1, BC, A], F32)
        for b in range(BC):
            pst = psC.tile([1, A], F32, tag="psc")
            eo = eo_tiles[b // 4]
            for p in range(2):
                nc.tensor.matmul(
                    out=pst,
                    lhsT=attT_q[:, 2 * p:2 * p + 2, b:b + 1],
                    rhs=eo[:, b % 4, 2 * p:2 * p + 2, :],
                    start=(p == 0), stop=(p == 1),
                    perf_mode=DR,
                )
            if b % 2 == 0:
                nc.scalar.copy(out=rows_sb[:, b, :], in_=pst)
            else:
                nc.vector.tensor_copy(out=rows_sb[:, b, :], in_=pst)
            if b % 8 == 7:
                nc.scalar.dma_start(out=ctx_sb[b - 7:b + 1, :],
                                    in_=rows_sb[:, b - 7:b + 1, :])

        # ctxT fp8 -> dec_inT slots 0-3 (x64 already)
        for c in range(4):
            ps_t = psT.tile([128, BC], F32, tag="tp")
            nc.tensor.transpose(ps_t[:AC, :], ctx_sb[:, c * AC:(c + 1) * AC],
                                ident[:BC, :BC])
            nc.scalar.copy(out=dec_inT[:AC, c, :], in_=ps_t[:AC, :])

        # ================= fc1: z1 = dec_in @ fc1_w^T (+bias) =================
        z1_sb = const.tile([BC, 1024], BF16)
        for nh in range(2):
            ps_z = psA.tile([BC, NT], F32, tag="mmA")
            ns = slice(nh * NT, (nh + 1) * NT)
            for p in range(2):  # ctx pairs, K=100
                nc.tensor.matmul(
                    out=ps_z,
                    lhsT=dec_inT[:AC, 2 * p:2 * p + 2, :],
                    rhs=fc1w_sb[:AC, 2 * p:2 * p + 2, ns],
                    start=(p == 0), stop=False, perf_mode=DR,
                )
            for p in range(2):  # h pairs, K=128
                nc.tensor.matmul(
                    out=ps_z,
                    lhsT=dec_inT[:, 4 + 2 * p:4 + 2 * p + 2, :],
                    rhs=fc1w_sb[:, 4 + 2 * p:4 + 2 * p + 2, ns],
                    start=False, stop=(p == 1), perf_mode=DR,
                )
            if with_fc1_bias:
                nc.vector.scalar_tensor_tensor(
                    out=z1_sb[:, ns], in0=ps_z, scalar=1.0 / (S_ATT * S_FC1W),
                    in1=fc1bias_sb[:, ns], op0=ALU.mult, op1=ALU.add,
                )
            else:
                nc.scalar.activation(out=z1_sb[:, ns], in_=ps_z, func=AF.Copy,
                                     scale=1.0 / (S_ATT * S_FC1W))

        # z1T fp8 (x16) [128, 8, 32] -> DRAM -> AllGather
        z1loc = const.tile([128, 8, BC], FP8)
        for j in range(8):
            ps_t = psT.tile([128, BC], BF16, tag="tp")
            nc.tensor.transpose(ps_t, z1_sb[:, j * 128:(j + 1) * 128],
                                ident_bf[:BC, :BC])
            nc.scalar.activation(out=z1loc[:, j, :], in_=ps_t, func=AF.Copy,
                                 scale=S_Z1)
        nc.scalar.dma_start(out=z1g_c[:], in_=z1loc)
        nc.gpsimd.collective_compute(
            "AllGather", ALU.bypass, replica_groups=RG,
            ins=[z1g_c.opt()], outs=[z1g_full.opt()],
        )

        # fc2 weights stream AFTER the collective trigger (gpsimd program
        # order) so pre-trigger HBM belongs to enco -> tighter core skew.
        fc2w_sb = const.tile([128, 8, VCP], FP8)
        for n0, nt in _vocab_tiles():
            nc.gpsimd.dma_start(
                out=fc2w_sb[:, :, n0:n0 + nt],
                in_=fc2wq[:].rearrange("p (k j) -> p k j", k=8)[:, :, n0:n0 + nt],
            )

        # ================= p_gen (all dots on PE, accumulate x128) =================
        ps_gen = psG.tile([1, BC], F32, tag="gen")
        for k in range(2):  # x0 . pg1  (pg1 x128, bf16)
            nc.tensor.matmul(out=ps_gen, lhsT=pg1_sb[:, k:k + 1], rhs=x0T_sb[:, k, :],
                             start=(k == 0), stop=False)
        for c in range(4):  # ctx . pg2 (ctxT x64 * pg2 x2, fp8)
            nc.tensor.matmul(out=ps_gen, lhsT=pg2_sb[:, c:c + 1],
                             rhs=dec_inT[:AC, c, :], start=False, stop=False)
        for c in range(4):  # h . pg3 (hT x64 * pg3 x2, fp8)
            nc.tensor.matmul(out=ps_gen, lhsT=pg3_sb[:, c:c + 1],
                             rhs=dec_inT[:, 4 + c, :], start=False, stop=(c == 3))
        gen_row = small.tile([1, BC], F32)
        nc.scalar.activation(out=gen_row, in_=ps_gen, func=AF.Sigmoid,
                             scale=1.0 / S_PG)
        nc.scalar.dma_start(out=gen_out[:], in_=gen_row)
        # gen column [32,1] via 1-k matmul with ones
        ps_gc = psT.tile([BC, 1], F32, tag="tp")
        nc.tensor.matmul(out=ps_gc, lhsT=gen_row, rhs=ones_sb)
        gen1m64 = small.tile([BC, 1], F32)
        nc.scalar.activation(out=gen1m64, in_=ps_gc, func=AF.Identity,
                             bias=c64_sb, scale=-1.0 / S_ATT)
        attcopy_sb = const.tile([BC, A], F32)
        nc.vector.tensor_scalar_mul(out=attcopy_sb, in0=att64_sb, scalar1=gen1m64)
        nc.scalar.dma_start(out=attcopy_out[:], in_=attcopy_sb)

        # ================= gathered z1T, two batch-half tiles =================
        # (separate tiles so fc2 on half 0 starts as soon as chunks 0-3 land)
        z1T_h0 = const.tile([128, 8, 128], FP8)
        z1T_h1 = const.tile([128, 8, 128], FP8)
        z1T_half = [z1T_h0, z1T_h1]
        for c in range(NCORES):
            nc.scalar.dma_start(
                out=z1T_half[c // 4][:, :, (c % 4) * BC:(c % 4 + 1) * BC],
                in_=z1g_full[c].rearrange("p (k b) -> p k b", k=8),
            )

        # ================= fc2 + exp + partial denominators =================
        s_acc = small.tile([128, 2], F32, tag="sacc")
        nc.vector.memset(s_acc, 0.0)
        for n0, nt in _vocab_tiles():
            nreal = min(nt, VC - n0)
            for bh in range(2):
                ps_l = psA.tile([128, NT], F32, tag="mmA")
                for p in range(4):
                    nc.tensor.matmul(
                        out=ps_l[:, :nt],
                        lhsT=z1T_half[bh][:, 2 * p:2 * p + 2, :],
                        rhs=fc2w_sb[:, 2 * p:2 * p + 2, n0:n0 + nt],
                        start=(p == 0), stop=(p == 3), perf_mode=DR,
                    )
                o_sb = op_.tile([128, NT], BF16, tag="osb")
                st = small.tile([128, 1], F32, tag="st")
                nc.scalar.activation(
                    out=o_sb[:, :nreal], in_=ps_l[:, :nreal], func=AF.Exp,
                    scale=1.0 / (S_Z1 * S_FC2W), accum_out=st,
                )
                nc.vector.tensor_add(out=s_acc[:, bh:bh + 1],
                                     in0=s_acc[:, bh:bh + 1], in1=st)
                nc.gpsimd.dma_start(
                    out=p_out[bh * 128:(bh + 1) * 128, n0:n0 + nreal],
                    in_=o_sb[:, :nreal],
                )
        nc.scalar.dma_start(out=s_out[:], in_=s_acc)

    nc.compile()
    return nc


_NC_CACHE = {}


def _get_nc(with_fc1_bias: bool) -> bass.Bass:
    if with_fc1_bias not in _NC_CACHE:
        _NC_CACHE[with_fc1_bias] = build_nc(with_fc1_bias)
    return _NC_CACHE[with_fc1_bias]


RUN_KW = {}        # test.py can set e.g. {"trace": True}
LAST_RESULT = {}   # test.py reads exec_time_ns etc.


def make_in_maps(inputs: dict):
    f32 = lambda a: np.ascontiguousarray(np.asarray(a), dtype=np.float32)
    bf16 = ml_dtypes.bfloat16
    fp8 = ml_dtypes.float8_e4m3  # TRN variant (max +-240)

    def q8(a):
        return np.clip(a, -240.0, 240.0).astype(fp8)

    x = f32(inputs["x"])[:, 0, :]              # [B, I]
    enco = f32(inputs["encoder_outputs"])      # [B, A, A]
    es = f32(inputs["encoder_state"])          # [B, H]
    W_ih = f32(inputs["W_ih"])                 # [4H, I]
    b = f32(inputs["b_ih"]) + f32(inputs["b_hh"])
    Wh_w = f32(inputs["Wh_w"])
    Ws_w = f32(inputs["Ws_w"])
    attb = f32(inputs["Wh_b"]) + f32(inputs["Ws_b"])
    vvec = f32(inputs["v"])
    fc1_w = f32(inputs["fc1_w"])               # [2H, H+A]
    fc1_b = f32(inputs["fc1_b"])
    fc2_w = f32(inputs["fc2_w"])               # [V, 2H]
    pg1 = f32(inputs["pg1_w"])[0]
    pg2 = f32(inputs["pg2_w"])[0]
    pg3 = f32(inputs["pg3_w"])[0]

    with_fc1_bias = bool(np.any(fc1_b != 0.0))

    # i, g, o gate rows (f gate dead: c0 = 0)
    idx = np.r_[0:H, 2 * H:3 * H, 3 * H:4 * H]
    wihT = np.ascontiguousarray(W_ih[idx].T).astype(bf16)   # [I, 1536]
    bgv = np.ascontiguousarray(b[idx])

    whsw = np.concatenate([Wh_w.T, Ws_w.T], axis=1)          # [H, 2A]
    smallp = np.concatenate([attb, vvec * S_ATT])            # [800]
    pg1b = (pg1 * S_PG).astype(bf16)
    pg2qv = q8(pg2 * 2.0)
    pg3qv = q8(pg3 * 2.0)

    # fc1 weights, prearranged in SBUF layout [128, 8, 1024] (x16)
    fc1T = fc1_w.T * S_FC1W                                  # [912, 1024]
    fc1wq = np.zeros((128, 8, 1024), np.float32)
    for k in range(4):
        fc1wq[:AC, k, :] = fc1T[k * AC:(k + 1) * AC]         # ctx chunks of 100
    for k in range(4):
        fc1wq[:, 4 + k, :] = fc1T[A + k * 128:A + (k + 1) * 128]
    fc1wq = q8(fc1wq.reshape(128, 8 * 1024))

    # fc2 weights [128, 8, 6256] (x32), cols >= 6250 zero
    fc2T = fc2_w.T * S_FC2W                                  # [1024, V]
    fc2wq_f = np.zeros((128, 8, VCP), np.float32)

    enco_pk = np.empty((AC, BC * 4 * A), ml_dtypes.float8_e4m3)

    x0T = np.ascontiguousarray(x.T)
    esTm = np.ascontiguousarray(es.T)

    in_maps = []
    enco_q8 = q8(enco)                                       # [B, A, A] fp8
    for c in range(NCORES):
        bs = slice(c * BC, (c + 1) * BC)
        vs = slice(c * VC, (c + 1) * VC)
        fc2wq_f[:, :, :] = 0.0
        for k in range(8):
            fc2wq_f[:, k, :VC] = fc2T[k * 128:(k + 1) * 128, vs]
        fc2wq = q8(fc2wq_f.reshape(128, 8 * VCP))
        # enco chunk-major: [p, (b, k, e)] = enco[bs][b, k*100+p, e]
        ec = enco_q8[bs]                                     # [32, 400, 400]
        epk = ec.reshape(BC, 4, AC, A).transpose(2, 0, 1, 3)  # [100, 32, 4, 400]
        enco_pk = np.ascontiguousarray(epk.reshape(AC, BC * 4 * A))
        m = {
            "x0T": np.ascontiguousarray(x0T[:, bs]).astype(bf16),
            "esT": np.ascontiguousarray(esTm[:, bs]).astype(bf16),
            "wihT": wihT,
            "bg": bgv,
            "whsw": whsw.astype(bf16),
            "smallp": smallp,
            "attb_bf": attb.astype(bf16),
            "pg1b": pg1b,
            "pg2q": pg2qv,
            "pg3q": pg3qv,
            "fc1wq": fc1wq,
            "fc2wq": fc2wq,
            "enco_q": enco_pk,
        }
        if with_fc1_bias:
            m["fc1bias"] = fc1_b
        in_maps.append(m)
    return in_maps, with_fc1_bias


def kernel(**inputs) -> np.ndarray:
    in_maps, with_fc1_bias = make_in_maps(inputs)
    nc = _get_nc(with_fc1_bias)

    res = run_bass_kernel_spmd(nc, in_maps, core_ids=list(range(NCORES)), **RUN_KW)
    results = res.results
    LAST_RESULT["exec_time_ns"] = getattr(res, "exec_time_ns", None)

    oov = int(np.asarray(inputs["max_oov_nums"]))
    ids = np.asarray(inputs["ids"])
    fc2_b = np.asarray(inputs["fc2_b"], dtype=np.float32)

    gen = np.concatenate(
        [np.asarray(results[c]["gen_out"])[0] for c in range(NCORES)])
    p = np.zeros((B, V + oov), dtype=np.float32)
    if np.any(fc2_b != 0.0):
        g = np.exp(fc2_b.astype(np.float64)).astype(np.float32)
        for c in range(NCORES):
            vs = slice(c * VC, (c + 1) * VC)
            p[:, vs] = np.asarray(results[c]["p_out"], dtype=np.float32) * g[None, vs]
        s = p[:, :V].sum(axis=1)
        p[:, :V] *= (gen / s)[:, None]
    else:
        s = np.zeros(B, dtype=np.float32)
        for c in range(NCORES):
            so = np.asarray(results[c]["s_out"])       # [128, 2]
            s += so.T.reshape(B)
        f = (gen / s).astype(np.float32)
        for c in range(NCORES):
            vs = slice(c * VC, (c + 1) * VC)
            p[:, vs] = np.asarray(results[c]["p_out"], dtype=np.float32) * f[:, None]

    att_copy = np.concatenate(
        [np.asarray(results[c]["attcopy_out"]) for c in range(NCORES)], axis=0)
    rows = np.arange(B)[:, None]
    np.add.at(p, (rows, ids), att_copy)
    return p


# revision 25
# speedup vs baseline: 1.3777x; 1.1542x over previous
"""Trainium2 Bass kernel for the pointer-generator decoder step (fp8 redesign).

Contract: kernel(**inputs) takes the FULL unsharded inputs and returns the
FULL [B, V+OOV] output.

Sharding (8 NeuronCores, one SPMD launch):
  * Front end (LSTM step, attention, context, p_gen, fc1) is data-parallel
    over batch (32 rows/core).
  * z1^T (fp8, 32 KB) is AllGathered so every core holds the full batch.
  * fc2 is tensor-parallel over vocab (6250 cols/core); exp(logits) and
    partial softmax denominators are computed on-chip; final normalization,
    the OOV extension and the copy scatter-add run on the host.

Precision: fp8(e4m3, TRN) for the context einsum (att x64), fc1 (inputs x64,
weights x16) and fc2 (z1 x16, weights x32) with DoubleRow perf mode;
fp32/f32r/bf16 elsewhere.  Host-simulated end-to-end rel err ~2.4e-3.
"""

import os
import sys

for _p in ("/opt/trn_rl_repo",):
    if _p not in sys.path and os.path.isdir(_p):
        sys.path.insert(0, _p)

import ml_dtypes
import numpy as np

import concourse.bass as bass
import concourse.bacc as bacc_mod
import concourse.mybir as mybir
import concourse.tile as tile
from concourse.bass_utils import run_bass_kernel_spmd
from concourse.masks import make_identity

NCORES = 8
B = 256           # batch
BC = B // NCORES  # batch shard per core (32)
I = 256           # input dim
H = 512           # hidden dim
A = 400           # attention dim
AC = 100          # a-chunk (4 chunks of 100 partitions)
V = 50000         # vocab
VC = V // NCORES  # vocab shard per core (6250)
VCP = 6256        # padded to %16 for DoubleRow stride
NT = 512          # vocab tile (one psum bank of fp32)

F32 = mybir.dt.float32
F32R = mybir.dt.float32r
BF16 = mybir.dt.bfloat16
FP8 = mybir.dt.float8e4
AF = mybir.ActivationFunctionType
ALU = mybir.AluOpType
AX = mybir.AxisListType
DR = mybir.MatmulPerfMode.DoubleRow

# scaling scheme (all powers of two, exact)
S_ATT = 64.0      # att, ctx, h carried x64 into fp8
S_FC1W = 16.0     # fc1 weights x16  -> z1 psum x1024
S_Z1 = 16.0       # z1 carried x16 into fp8
S_FC2W = 32.0     # fc2 weights x32  -> logits psum x512
S_PG = 128.0      # p_gen dot products accumulate x128


def _bc(ap, parts):
    """Broadcast a DRAM AP across `parts` partitions (0-stride partition dim)."""
    return bass.AP(tensor=ap.tensor, offset=ap.offset, ap=[[0, parts]] + list(ap.ap))


def _pstride(ap, stride, num):
    """Partition-strided view of a PSUM/SBUF AP (rows 0, stride, 2*stride...)."""
    return bass.AP(tensor=ap.tensor, offset=ap.offset,
                   ap=[[stride, num]] + list(ap.ap)[1:])


def _vocab_tiles():
    out = []
    n0 = 0
    while n0 < VCP:
        out.append((n0, min(NT, VCP - n0)))
        n0 += NT
    return out


def build_nc(with_fc1_bias: bool) -> bass.Bass:
    nc = bacc_mod.Bacc("TRN2", target_bir_lowering=False, num_devices=NCORES)

    # ---- external inputs ----
    x0T = nc.dram_tensor("x0T", [I, BC], BF16, kind="ExternalInput")
    esT = nc.dram_tensor("esT", [H, BC], BF16, kind="ExternalInput")
    wihT = nc.dram_tensor("wihT", [I, 12 * 128], BF16, kind="ExternalInput")
    bg = nc.dram_tensor("bg", [12 * 128], F32, kind="ExternalInput")
    whsw = nc.dram_tensor("whsw", [H, 2 * A], BF16, kind="ExternalInput")
    smallp = nc.dram_tensor("smallp", [2 * A], F32, kind="ExternalInput")  # attb|v64
    attb_bf = nc.dram_tensor("attb_bf", [A], BF16, kind="ExternalInput")
    pg1b = nc.dram_tensor("pg1b", [I], BF16, kind="ExternalInput")         # x128
    pg2q = nc.dram_tensor("pg2q", [A], FP8, kind="ExternalInput")          # x2
    pg3q = nc.dram_tensor("pg3q", [H], FP8, kind="ExternalInput")          # x2
    fc1wq = nc.dram_tensor("fc1wq", [128, 8 * 1024], FP8, kind="ExternalInput")
    fc2wq = nc.dram_tensor("fc2wq", [128, 8 * VCP], FP8, kind="ExternalInput")
    enco_q = nc.dram_tensor("enco_q", [AC, BC * 4 * A], FP8, kind="ExternalInput")
    if with_fc1_bias:
        fc1bias = nc.dram_tensor("fc1bias", [1024], F32, kind="ExternalInput")

    # ---- external outputs ----
    p_out = nc.dram_tensor("p_out", [B, VC], BF16, kind="ExternalOutput")
    s_out = nc.dram_tensor("s_out", [128, 2], F32, kind="ExternalOutput")
    attcopy_out = nc.dram_tensor("attcopy_out", [BC, A], F32, kind="ExternalOutput")
    gen_out = nc.dram_tensor("gen_out", [1, BC], F32, kind="ExternalOutput")

    RG = [list(range(NCORES))]

    from contextlib import ExitStack

    with tile.TileContext(nc) as tc, ExitStack() as ctx:
        dram = ctx.enter_context(tc.tile_pool(name="dram", bufs=1, space="DRAM"))
        z1g_c = dram.tile([128, 8 * BC], FP8)                    # z1T local [128,8k,32b]
        z1g_full = dram.tile([NCORES, 128, 8 * BC], FP8, addr_space="Shared")

        const = ctx.enter_context(tc.tile_pool(name="const", bufs=1))
        small = ctx.enter_context(tc.tile_pool(name="small", bufs=4))
        eop = ctx.enter_context(tc.tile_pool(name="eop", bufs=8))
        op_ = ctx.enter_context(tc.tile_pool(name="op", bufs=3))
        psA = ctx.enter_context(tc.tile_pool(name="psA", bufs=3, space="PSUM"))
        psC = ctx.enter_context(tc.tile_pool(name="psC", bufs=2, space="PSUM"))
        psT = ctx.enter_context(tc.tile_pool(name="psT", bufs=2, space="PSUM"))
        psG = ctx.enter_context(tc.tile_pool(name="psG", bufs=1, space="PSUM"))

        ident = const.tile([128, 128], F32)
        make_identity(nc, ident)
        ident_bf = const.tile([128, 128], BF16)
        make_identity(nc, ident_bf)

        # ================= DMA issue plan =================
        # sync (SP HWDGE): LSTM/attention consts, then enco (critical), then
        #   fc1w/fc2w bulk - FIFO order on SP's queue gives enco priority.
        # scalar (Act HWDGE): latency-critical small DMAs (ctx psum gathers,
        #   z1 store, z1T gathered loads) - interleaves with SP's bulk.
        # gpsimd (SWDGE): tiny consts, p_out stores, collective trigger.
        x0T_sb = const.tile([128, 2, BC], BF16)
        nc.sync.dma_start(out=x0T_sb, in_=x0T[:].rearrange("(k p) b -> p k b", p=128))
        wih_sb = const.tile([128, 2, 12 * 128], BF16)
        nc.sync.dma_start(out=wih_sb, in_=wihT[:].rearrange("(k p) m -> p k m", p=128))
        esT_sb = const.tile([128, 4, BC], BF16)
        nc.sync.dma_start(out=esT_sb, in_=esT[:].rearrange("(k p) b -> p k b", p=128))
        whsw_sb = const.tile([128, 4, 2 * A], BF16)
        nc.sync.dma_start(out=whsw_sb, in_=whsw[:].rearrange("(k p) a -> p k a", p=128))

        # fc1w on the Act queue: interleaves with SP's enco stream, in early
        fc1w_sb = const.tile([128, 8, 1024], FP8)
        nc.scalar.dma_start(out=fc1w_sb, in_=fc1wq[:].rearrange("p (k m) -> p k m", k=8))

        # enco: 8 groups of 4 batch rows, [100, 4b, 4k, 400] fp8 each
        eo_tiles = []
        for g in range(8):
            eo = eop.tile([AC, 4, 4, A], FP8, tag=f"eo{g}", bufs=1)
            nc.sync.dma_start(
                out=eo,
                in_=enco_q[:, g * 4 * 4 * A:(g + 1) * 4 * 4 * A]
                .rearrange("p (b k e) -> p b k e", b=4, k=4),
            )
            eo_tiles.append(eo)

        # fc2 weights on the SAME SP queue AFTER enco: per-queue FIFO gives
        # enco strict priority (the tile scheduler reorders across queues,
        # so cross-engine "late issue" does not work).
        fc2w_sb = const.tile([128, 8, VCP], FP8)
        for n0, nt in _vocab_tiles():
            nc.sync.dma_start(
                out=fc2w_sb[:, :, n0:n0 + nt],
                in_=fc2wq[:].rearrange("p (k j) -> p k j", k=8)[:, :, n0:n0 + nt],
            )

        # tiny consts on gpsimd
        bg_sb = const.tile([128, 12], F32)
        nc.gpsimd.dma_start(out=bg_sb, in_=bg[:].rearrange("(m p) -> p m", p=128))
        smallc = const.tile([BC, 2 * A], F32)
        nc.gpsimd.dma_start(out=smallc, in_=_bc(smallp[:], BC))
        attb_sb = smallc[:, 0:A]
        v64_sb = smallc[:, A:2 * A]
        pg1_sb = const.tile([128, 2], BF16)
        nc.gpsimd.dma_start(out=pg1_sb, in_=pg1b[:].rearrange("(k p) -> p k", p=128))
        pg2_sb = const.tile([AC, 4], FP8)
        nc.gpsimd.dma_start(out=pg2_sb, in_=pg2q[:].rearrange("(k p) -> p k", p=AC))
        pg3_sb = const.tile([128, 4], FP8)
        nc.gpsimd.dma_start(out=pg3_sb, in_=pg3q[:].rearrange("(k p) -> p k", p=128))
        if with_fc1_bias:
            fc1bias_sb = const.tile([BC, 1024], F32)
            nc.gpsimd.dma_start(out=fc1bias_sb, in_=_bc(fc1bias[:], BC))

        attbr_sb = const.tile([1, A], BF16)
        nc.gpsimd.dma_start(out=attbr_sb, in_=attb_bf[:].rearrange("(p a) -> p a", p=1))
        ones_sb = small.tile([1, 1], F32)
        nc.vector.memset(ones_sb, 1.0)
        onesb_sb = small.tile([1, BC], BF16)
        nc.vector.memset(onesb_sb, 1.0)
        c64_sb = small.tile([BC, 1], F32)
        nc.vector.memset(c64_sb, 1.0 / S_ATT)

        # ================= LSTM step (h only) =================
        sg_sb = const.tile([128, 12, BC], F32)  # sig(i), tanh(g), sig(o)
        for m in range(12):
            ps_g = psA.tile([128, BC], F32, tag="mmA")
            for k in range(2):
                nc.tensor.matmul(
                    out=ps_g,
                    lhsT=wih_sb[:, k, m * 128:(m + 1) * 128],
                    rhs=x0T_sb[:, k, :],
                    start=(k == 0), stop=(k == 1),
                )
            func = AF.Tanh if 4 <= m < 8 else AF.Sigmoid
            nc.scalar.activation(
                out=sg_sb[:, m, :], in_=ps_g, func=func,
                bias=bg_sb[:, m:m + 1], scale=1.0,
            )
        cth_sb = const.tile([128, 4, BC], F32)
        nc.vector.tensor_mul(out=cth_sb, in0=sg_sb[:, 0:4, :], in1=sg_sb[:, 4:8, :])
        nc.scalar.activation(out=cth_sb, in_=cth_sb, func=AF.Tanh)
        hT_sb = const.tile([128, 4, BC], BF16)  # h feature-major (attention lhsT)
        nc.vector.tensor_mul(out=hT_sb, in0=sg_sb[:, 8:12, :], in1=cth_sb)

        # dec_inT fp8 [128, 8, 32]: slots 0-3 ctx x64 (100 partitions), 4-7 h x64
        dec_inT = const.tile([128, 8, BC], FP8)
        nc.scalar.activation(out=dec_inT[:, 4:8, :], in_=hT_sb,
                             func=AF.Copy, scale=S_ATT)

        # ================= attention =================
        # e psum accumulates es@Wh + h@Ws + attb (attb via a K=1 ones matmul);
        # tanh bounds e, so softmax needs no max subtraction.
        ps_e = psA.tile([BC, A], F32, tag="mmA")
        for k in range(4):
            nc.tensor.matmul(out=ps_e, lhsT=esT_sb[:, k, :], rhs=whsw_sb[:, k, 0:A],
                             start=(k == 0), stop=False)
        for k in range(4):
            nc.tensor.matmul(out=ps_e, lhsT=hT_sb[:, k, :], rhs=whsw_sb[:, k, A:2 * A],
                             start=False, stop=False)
        nc.tensor.matmul(out=ps_e, lhsT=onesb_sb, rhs=attbr_sb,
                         start=False, stop=True)
        e_sb = const.tile([BC, A], F32)
        nc.scalar.activation(out=e_sb, in_=ps_e, func=AF.Tanh)
        ssum = small.tile([BC, 1], F32)
        nc.scalar.activation(out=e_sb, in_=e_sb, func=AF.Exp, accum_out=ssum)
        rs = small.tile([BC, 1], F32)
        nc.vector.reciprocal(out=rs, in_=ssum)
        att64_sb = const.tile([BC, A], F32)  # att x 64
        nc.vector.scalar_tensor_tensor(out=att64_sb, in0=e_sb, scalar=rs, in1=v64_sb,
                                       op0=ALU.mult, op1=ALU.mult)

        # attT fp8 [100p, 4, 32] via PE transposes (chunks of 100)
        attT_q = const.tile([AC, 4, BC], FP8)
        for c in range(4):
            ps_t = psT.tile([128, BC], F32, tag="tp")
            nc.tensor.transpose(ps_t[:AC, :], att64_sb[:, c * AC:(c + 1) * AC],
                                ident[:BC, :BC])
            nc.scalar.copy(out=attT_q[:, c, :], in_=ps_t[:AC, :])

        # ================= context: ctx64[b,:] = att64[b] @ enco[b] =================
        # Per-row [1,400] psum results; psum->SBUF copies (partition 0 scratch,
        # alternating Scalar/Vector) hide under the enco DMA; 4 chunked
        # SBUF->SBUF DMAs restore batch-major layout.
        ctx_sb = const.tile([BC, A], F32)  # ctx x 64, batch-major
        rows_sb = const.tile([1, BC, A], F32)
        for b in range(BC):
            pst = psC.tile([1, A], F32, tag="psc")
            eo = eo_tiles[b // 4]
            for p in range(2):
                nc.tensor.matmul(
                    out=pst,
                    lhsT=attT_q[:, 2 * p:2 * p + 2, b:b + 1],
                    rhs=eo[:, b % 4, 2 * p:2 * p + 2, :],
                    start=(p == 0), stop=(p == 1),
                    perf_mode=DR,
                )
            if b % 2 == 0:
                nc.scalar.copy(out=rows_sb[:, b, :], in_=pst)
            else:
                nc.vector.tensor_copy(out=rows_sb[:, b, :], in_=pst)
            if b % 8 == 7:
                nc.scalar.dma_start(out=ctx_sb[b - 7:b + 1, :],
                                    in_=rows_sb[:, b - 7:b + 1, :])

        # ctxT fp8 -> dec_inT slots 0-3 (x64 already)
        for c in range(4):
            ps_t = psT.tile([128, BC], F32, tag="tp")
            nc.tensor.transpose(ps_t[:AC, :], ctx_sb[:, c * AC:(c + 1) * AC],
                                ident[:BC, :BC])
            nc.scalar.copy(out=dec_inT[:AC, c, :], in_=ps_t[:AC, :])

        # ================= fc1: z1 = dec_in @ fc1_w^T (+bias) =================
        z1_sb = const.tile([BC, 1024], BF16)
        for nh in range(2):
            ps_z = psA.tile([BC, NT], F32, tag="mmA")
            ns = slice(nh * NT, (nh + 1) * NT)
            for p in range(2):  # ctx pairs, K=100
                nc.tensor.matmul(
                    out=ps_z,
                    lhsT=dec_inT[:AC, 2 * p:2 * p + 2, :],
                    rhs=fc1w_sb[:AC, 2 * p:2 * p + 2, ns],
                    start=(p == 0), stop=False, perf_mode=DR,
                )
            for p in range(2):  # h pairs, K=128
                nc.tensor.matmul(
                    out=ps_z,
                    lhsT=dec_inT[:, 4 + 2 * p:4 + 2 * p + 2, :],
                    rhs=fc1w_sb[:, 4 + 2 * p:4 + 2 * p + 2, ns],
                    start=False, stop=(p == 1), perf_mode=DR,
                )
            if with_fc1_bias:
                nc.vector.scalar_tensor_tensor(
                    out=z1_sb[:, ns], in0=ps_z, scalar=1.0 / (S_ATT * S_FC1W),
                    in1=fc1bias_sb[:, ns], op0=ALU.mult, op1=ALU.add,
                )
            else:
                nc.scalar.activation(out=z1_sb[:, ns], in_=ps_z, func=AF.Copy,
                                     scale=1.0 / (S_ATT * S_FC1W))

        # z1T fp8 (x16) [128, 8, 32] -> DRAM -> AllGather
        z1loc = const.tile([128, 8, BC], FP8)
        for j in range(8):
            ps_t = psT.tile([128, BC], BF16, tag="tp")
            nc.tensor.transpose(ps_t, z1_sb[:, j * 128:(j + 1) * 128],
                                ident_bf[:BC, :BC])
            nc.scalar.activation(out=z1loc[:, j, :], in_=ps_t, func=AF.Copy,
                                 scale=S_Z1)
        nc.scalar.dma_start(out=z1g_c[:], in_=z1loc)
        nc.gpsimd.collective_compute(
            "AllGather", ALU.bypass, replica_groups=RG,
            ins=[z1g_c.opt()], outs=[z1g_full.opt()],
        )

        # ================= gathered z1T, two batch-half tiles =================
        # Load each core's chunk in its native layout (contiguous 256B per
        # partition -> 128 descriptors per DMA instead of 1024x32B), then
        # engine-copy into the k-major z1T layout the DR matmuls need.
        z1cat = const.tile([128, NCORES, 8, BC], FP8)
        for c in range(NCORES):
            nc.scalar.dma_start(out=z1cat[:, c, :, :], in_=z1g_full[c])
        z1T_h0 = const.tile([128, 8, 128], FP8)
        z1T_h1 = const.tile([128, 8, 128], FP8)
        z1T_half = [z1T_h0, z1T_h1]
        for c in range(NCORES):
            dst = z1T_half[c // 4][:, :, (c % 4) * BC:(c % 4 + 1) * BC]
            if c % 2 == 0:
                nc.scalar.copy(out=dst, in_=z1cat[:, c, :, :])
            else:
                nc.vector.tensor_copy(out=dst, in_=z1cat[:, c, :, :])

        # ================= fc2 + exp + partial denominators =================
        s_acc = small.tile([128, 2], F32, tag="sacc")
        nc.vector.memset(s_acc, 0.0)
        for n0, nt in _vocab_tiles():
            nreal = min(nt, VC - n0)
            for bh in range(2):
                ps_l = psA.tile([128, NT], F32, tag="mmA")
                for p in range(4):
                    nc.tensor.matmul(
                        out=ps_l[:, :nt],
                        lhsT=z1T_half[bh][:, 2 * p:2 * p + 2, :],
                        rhs=fc2w_sb[:, 2 * p:2 * p + 2, n0:n0 + nt],
                        start=(p == 0), stop=(p == 3), perf_mode=DR,
                    )
                o_sb = op_.tile([128, NT], BF16, tag="osb")
                st = small.tile([128, 1], F32, tag="st")
                nc.scalar.activation(
                    out=o_sb[:, :nreal], in_=ps_l[:, :nreal], func=AF.Exp,
                    scale=1.0 / (S_Z1 * S_FC2W), accum_out=st,
                )
                nc.vector.tensor_add(out=s_acc[:, bh:bh + 1],
                                     in0=s_acc[:, bh:bh + 1], in1=st)
                nc.gpsimd.dma_start(
                    out=p_out[bh * 128:(bh + 1) * 128, n0:n0 + nreal],
                    in_=o_sb[:, :nreal],
                )
        nc.scalar.dma_start(out=s_out[:], in_=s_acc)

        # ================= p_gen (off the critical AG path; runs under fc2) ==
        ps_gen = psG.tile([1, BC], F32, tag="gen")
        for k in range(2):  # x0 . pg1  (pg1 x128, bf16)
            nc.tensor.matmul(out=ps_gen, lhsT=pg1_sb[:, k:k + 1], rhs=x0T_sb[:, k, :],
                             start=(k == 0), stop=False)
        for c in range(4):  # ctx . pg2 (ctxT x64 * pg2 x2, fp8)
            nc.tensor.matmul(out=ps_gen, lhsT=pg2_sb[:, c:c + 1],
                             rhs=dec_inT[:AC, c, :], start=False, stop=False)
        for c in range(4):  # h . pg3 (hT x64 * pg3 x2, fp8)
            nc.tensor.matmul(out=ps_gen, lhsT=pg3_sb[:, c:c + 1],
                             rhs=dec_inT[:, 4 + c, :], start=False, stop=(c == 3))
        gen_row = small.tile([1, BC], F32)
        nc.scalar.activation(out=gen_row, in_=ps_gen, func=AF.Sigmoid,
                             scale=1.0 / S_PG)
        nc.scalar.dma_start(out=gen_out[:], in_=gen_row)
        # gen column [32,1] via 1-k matmul with ones
        ps_gc = psT.tile([BC, 1], F32, tag="tp")
        nc.tensor.matmul(out=ps_gc, lhsT=gen_row, rhs=ones_sb)
        gen1m64 = small.tile([BC, 1], F32)
        nc.scalar.activation(out=gen1m64, in_=ps_gc, func=AF.Identity,
                             bias=c64_sb, scale=-1.0 / S_ATT)
        attcopy_sb = const.tile([BC, A], F32)
        nc.vector.tensor_scalar_mul(out=attcopy_sb, in0=att64_sb, scalar1=gen1m64)
        nc.scalar.dma_start(out=attcopy_out[:], in_=attcopy_sb)

    nc.compile()
    return nc


_NC_CACHE = {}


def _get_nc(with_fc1_bias: bool) -> bass.Bass:
    if with_fc1_bias not in _NC_CACHE:
        _NC_CACHE[with_fc1_bias] = build_nc(with_fc1_bias)
    return _NC_CACHE[with_fc1_bias]


RUN_KW = {}        # test.py can set e.g. {"trace": True}
LAST_RESULT = {}   # test.py reads exec_time_ns etc.


def make_in_maps(inputs: dict):
    f32 = lambda a: np.ascontiguousarray(np.asarray(a), dtype=np.float32)
    bf16 = ml_dtypes.bfloat16
    fp8 = ml_dtypes.float8_e4m3  # TRN variant (max +-240)

    def q8(a):
        return np.clip(a, -240.0, 240.0).astype(fp8)

    x = f32(inputs["x"])[:, 0, :]              # [B, I]
    enco = f32(inputs["encoder_outputs"])      # [B, A, A]
    es = f32(inputs["encoder_state"])          # [B, H]
    W_ih = f32(inputs["W_ih"])                 # [4H, I]
    b = f32(inputs["b_ih"]) + f32(inputs["b_hh"])
    Wh_w = f32(inputs["Wh_w"])
    Ws_w = f32(inputs["Ws_w"])
    attb = f32(inputs["Wh_b"]) + f32(inputs["Ws_b"])
    vvec = f32(inputs["v"])
    fc1_w = f32(inputs["fc1_w"])               # [2H, H+A]
    fc1_b = f32(inputs["fc1_b"])
    fc2_w = f32(inputs["fc2_w"])               # [V, 2H]
    pg1 = f32(inputs["pg1_w"])[0]
    pg2 = f32(inputs["pg2_w"])[0]
    pg3 = f32(inputs["pg3_w"])[0]

    with_fc1_bias = bool(np.any(fc1_b != 0.0))

    # i, g, o gate rows (f gate dead: c0 = 0)
    idx = np.r_[0:H, 2 * H:3 * H, 3 * H:4 * H]
    wihT = np.ascontiguousarray(W_ih[idx].T).astype(bf16)   # [I, 1536]
    bgv = np.ascontiguousarray(b[idx])

    whsw = np.concatenate([Wh_w.T, Ws_w.T], axis=1)          # [H, 2A]
    smallp = np.concatenate([attb, vvec * S_ATT])            # [800]
    pg1b = (pg1 * S_PG).astype(bf16)
    pg2qv = q8(pg2 * 2.0)
    pg3qv = q8(pg3 * 2.0)

    # fc1 weights, prearranged in SBUF layout [128, 8, 1024] (x16)
    fc1T = fc1_w.T * S_FC1W                                  # [912, 1024]
    fc1wq = np.zeros((128, 8, 1024), np.float32)
    for k in range(4):
        fc1wq[:AC, k, :] = fc1T[k * AC:(k + 1) * AC]         # ctx chunks of 100
    for k in range(4):
        fc1wq[:, 4 + k, :] = fc1T[A + k * 128:A + (k + 1) * 128]
    fc1wq = q8(fc1wq.reshape(128, 8 * 1024))

    # fc2 weights [128, 8, 6256] (x32), cols >= 6250 zero
    fc2T = fc2_w.T * S_FC2W                                  # [1024, V]
    fc2wq_f = np.zeros((128, 8, VCP), np.float32)

    enco_pk = np.empty((AC, BC * 4 * A), ml_dtypes.float8_e4m3)

    x0T = np.ascontiguousarray(x.T)
    esTm = np.ascontiguousarray(es.T)

    in_maps = []
    enco_q8 = q8(enco)                                       # [B, A, A] fp8
    for c in range(NCORES):
        bs = slice(c * BC, (c + 1) * BC)
        vs = slice(c * VC, (c + 1) * VC)
        fc2wq_f[:, :, :] = 0.0
        for k in range(8):
            fc2wq_f[:, k, :VC] = fc2T[k * 128:(k + 1) * 128, vs]
        fc2wq = q8(fc2wq_f.reshape(128, 8 * VCP))
        # enco chunk-major: [p, (b, k, e)] = enco[bs][b, k*100+p, e]
        ec = enco_q8[bs]                                     # [32, 400, 400]
        epk = ec.reshape(BC, 4, AC, A).transpose(2, 0, 1, 3)  # [100, 32, 4, 400]
        enco_pk = np.ascontiguousarray(epk.reshape(AC, BC * 4 * A))
        m = {
            "x0T": np.ascontiguousarray(x0T[:, bs]).astype(bf16),
            "esT": np.ascontiguousarray(esTm[:, bs]).astype(bf16),
            "wihT": wihT,
            "bg": bgv,
            "whsw": whsw.astype(bf16),
            "smallp": smallp,
            "attb_bf": attb.astype(bf16),
            "pg1b": pg1b,
            "pg2q": pg2qv,
            "pg3q": pg3qv,
            "fc1wq": fc1wq,
            "fc2wq": fc2wq,
            "enco_q": enco_pk,
        }
        if with_fc1_bias:
            m["fc1bias"] = fc1_b
        in_maps.append(m)
    return in_maps, with_fc1_bias


def kernel(**inputs) -> np.ndarray:
    in_maps, with_fc1_bias = make_in_maps(inputs)
    nc = _get_nc(with_fc1_bias)

    res = run_bass_kernel_spmd(nc, in_maps, core_ids=list(range(NCORES)), **RUN_KW)
    results = res.results
    LAST_RESULT["exec_time_ns"] = getattr(res, "exec_time_ns", None)

    oov = int(np.asarray(inputs["max_oov_nums"]))
    ids = np.asarray(inputs["ids"])
    fc2_b = np.asarray(inputs["fc2_b"], dtype=np.float32)

    gen = np.concatenate(
        [np.asarray(results[c]["gen_out"])[0] for c in range(NCORES)])
    p = np.zeros((B, V + oov), dtype=np.float32)
    if np.any(fc2_b != 0.0):
        g = np.exp(fc2_b.astype(np.float64)).astype(np.float32)
        for c in range(NCORES):
            vs = slice(c * VC, (c + 1) * VC)
            p[:, vs] = np.asarray(results[c]["p_out"], dtype=np.float32) * g[None, vs]
        s = p[:, :V].sum(axis=1)
        p[:, :V] *= (gen / s)[:, None]
    else:
        s = np.zeros(B, dtype=np.float32)
        for c in range(NCORES):
            so = np.asarray(results[c]["s_out"])       # [128, 2]
            s += so.T.reshape(B)
        f = (gen / s).astype(np.float32)
        for c in range(NCORES):
            vs = slice(c * VC, (c + 1) * VC)
            p[:, vs] = np.asarray(results[c]["p_out"], dtype=np.float32) * f[:, None]

    att_copy = np.concatenate(
        [np.asarray(results[c]["attcopy_out"]) for c in range(NCORES)], axis=0)
    rows = np.arange(B)[:, None]
    np.add.at(p, (rows, ids), att_copy)
    return p


# revision 29
# speedup vs baseline: 1.4475x; 1.0506x over previous
"""Trainium2 Bass kernel for the pointer-generator decoder step (fp8 redesign).

Contract: kernel(**inputs) takes the FULL unsharded inputs and returns the
FULL [B, V+OOV] output.

Sharding (8 NeuronCores, one SPMD launch):
  * Front end (LSTM step, attention, context, p_gen, fc1) is data-parallel
    over batch (32 rows/core).
  * z1^T (fp8, 32 KB) is AllGathered so every core holds the full batch.
  * fc2 is tensor-parallel over vocab (6250 cols/core); exp(logits) and
    partial softmax denominators are computed on-chip; final normalization,
    the OOV extension and the copy scatter-add run on the host.

Precision: fp8(e4m3, TRN) for the context einsum (att x64), fc1 (inputs x64,
weights x16) and fc2 (z1 x16, weights x32) with DoubleRow perf mode;
fp32/f32r/bf16 elsewhere.  Host-simulated end-to-end rel err ~2.4e-3.
"""

import os
import sys

for _p in ("/opt/trn_rl_repo",):
    if _p not in sys.path and os.path.isdir(_p):
        sys.path.insert(0, _p)

import ml_dtypes
import numpy as np

import concourse.bass as bass
import concourse.bacc as bacc_mod
import concourse.mybir as mybir
import concourse.tile as tile
from concourse.bass_utils import run_bass_kernel_spmd
from concourse.masks import make_identity

NCORES = 8
B = 256           # batch
BC = B // NCORES  # batch shard per core (32)
I = 256           # input dim
H = 512           # hidden dim
A = 400           # attention dim
AC = 100          # a-chunk (4 chunks of 100 partitions)
V = 50000         # vocab
VC = V // NCORES  # vocab shard per core (6250)
VCP = 6256        # padded to %16 for DoubleRow stride
NT = 512          # vocab tile (one psum bank of fp32)

F32 = mybir.dt.float32
F32R = mybir.dt.float32r
BF16 = mybir.dt.bfloat16
FP8 = mybir.dt.float8e4
AF = mybir.ActivationFunctionType
ALU = mybir.AluOpType
AX = mybir.AxisListType
DR = mybir.MatmulPerfMode.DoubleRow

# scaling scheme (all powers of two, exact)
S_ATT = 64.0      # att, ctx, h carried x64 into fp8
S_FC1W = 16.0     # fc1 weights x16  -> z1 psum x1024
S_Z1 = 16.0       # z1 carried x16 into fp8
S_FC2W = 32.0     # fc2 weights x32  -> logits psum x512
S_PG = 128.0      # p_gen dot products accumulate x128


def _bc(ap, parts):
    """Broadcast a DRAM AP across `parts` partitions (0-stride partition dim)."""
    return bass.AP(tensor=ap.tensor, offset=ap.offset, ap=[[0, parts]] + list(ap.ap))


def _pstride(ap, stride, num):
    """Partition-strided view of a PSUM/SBUF AP (rows 0, stride, 2*stride...)."""
    return bass.AP(tensor=ap.tensor, offset=ap.offset,
                   ap=[[stride, num]] + list(ap.ap)[1:])


def _vocab_tiles():
    out = []
    n0 = 0
    while n0 < VCP:
        out.append((n0, min(NT, VCP - n0)))
        n0 += NT
    return out


def build_nc(with_fc1_bias: bool) -> bass.Bass:
    nc = bacc_mod.Bacc("TRN2", target_bir_lowering=False, num_devices=NCORES)

    # ---- external inputs ----
    x0T = nc.dram_tensor("x0T", [I, BC], BF16, kind="ExternalInput")
    esT = nc.dram_tensor("esT", [H, BC], BF16, kind="ExternalInput")
    wihT = nc.dram_tensor("wihT", [I, 12 * 128], BF16, kind="ExternalInput")
    bg = nc.dram_tensor("bg", [12 * 128], F32, kind="ExternalInput")
    whsw = nc.dram_tensor("whsw", [H, 2 * A], BF16, kind="ExternalInput")
    smallp = nc.dram_tensor("smallp", [2 * A], F32, kind="ExternalInput")  # attb|v64
    attb_bf = nc.dram_tensor("attb_bf", [A], BF16, kind="ExternalInput")
    pg1b = nc.dram_tensor("pg1b", [I], BF16, kind="ExternalInput")         # x128
    pg2q = nc.dram_tensor("pg2q", [A], FP8, kind="ExternalInput")          # x2
    pg3q = nc.dram_tensor("pg3q", [H], FP8, kind="ExternalInput")          # x2
    fc1wq = nc.dram_tensor("fc1wq", [128, 8 * 1024], FP8, kind="ExternalInput")
    fc2wq = nc.dram_tensor("fc2wq", [128, 8 * VCP], FP8, kind="ExternalInput")
    enco_q = nc.dram_tensor("enco_q", [AC, BC * 4 * A], FP8, kind="ExternalInput")
    if with_fc1_bias:
        fc1bias = nc.dram_tensor("fc1bias", [1024], F32, kind="ExternalInput")

    # ---- external outputs ----
    p_out = nc.dram_tensor("p_out", [B, VC], BF16, kind="ExternalOutput")
    s_out = nc.dram_tensor("s_out", [128, 2], F32, kind="ExternalOutput")
    attcopy_out = nc.dram_tensor("attcopy_out", [BC, A], F32, kind="ExternalOutput")
    gen_out = nc.dram_tensor("gen_out", [1, BC], F32, kind="ExternalOutput")

    RG = [list(range(NCORES))]

    from contextlib import ExitStack

    with tile.TileContext(nc) as tc, ExitStack() as ctx:
        dram = ctx.enter_context(tc.tile_pool(name="dram", bufs=1, space="DRAM"))
        z1g_c = dram.tile([128, 8 * BC], FP8)                    # z1T local [128,8k,32b]
        z1g_full = dram.tile([NCORES, 128, 8 * BC], FP8, addr_space="Shared")

        const = ctx.enter_context(tc.tile_pool(name="const", bufs=1))
        small = ctx.enter_context(tc.tile_pool(name="small", bufs=4))
        eop = ctx.enter_context(tc.tile_pool(name="eop", bufs=8))
        op_ = ctx.enter_context(tc.tile_pool(name="op", bufs=3))
        psA = ctx.enter_context(tc.tile_pool(name="psA", bufs=2, space="PSUM"))
        psC = ctx.enter_context(tc.tile_pool(name="psC", bufs=3, space="PSUM"))
        psT = ctx.enter_context(tc.tile_pool(name="psT", bufs=2, space="PSUM"))
        psG = ctx.enter_context(tc.tile_pool(name="psG", bufs=1, space="PSUM"))

        ident = const.tile([128, 128], F32)
        make_identity(nc, ident)
        ident_bf = const.tile([128, 128], BF16)
        make_identity(nc, ident_bf)

        # ================= DMA issue plan =================
        # sync (SP HWDGE): LSTM/attention consts, then enco (critical), then
        #   fc1w/fc2w bulk - FIFO order on SP's queue gives enco priority.
        # scalar (Act HWDGE): latency-critical small DMAs (ctx psum gathers,
        #   z1 store, z1T gathered loads) - interleaves with SP's bulk.
        # gpsimd (SWDGE): tiny consts, p_out stores, collective trigger.
        x0T_sb = const.tile([128, 2, BC], BF16)
        nc.sync.dma_start(out=x0T_sb, in_=x0T[:].rearrange("(k p) b -> p k b", p=128))
        esT_sb = const.tile([128, 4, BC], BF16)
        nc.sync.dma_start(out=esT_sb, in_=esT[:].rearrange("(k p) b -> p k b", p=128))
        whsw_sb = const.tile([128, 4, 2 * A], BF16)
        nc.sync.dma_start(out=whsw_sb, in_=whsw[:].rearrange("(k p) a -> p k a", p=128))
        wih_sb = const.tile([128, 2, 12 * 128], BF16)
        nc.sync.dma_start(out=wih_sb, in_=wihT[:].rearrange("(k p) m -> p k m", p=128))

        # fc1w on the Act queue: interleaves with SP's enco stream, in early
        fc1w_sb = const.tile([128, 8, 1024], FP8)
        nc.scalar.dma_start(out=fc1w_sb, in_=fc1wq[:].rearrange("p (k m) -> p k m", k=8))

        # enco: 8 groups of 4 batch rows, [100, 4b, 4k, 400] fp8 each
        eo_tiles = []
        for g in range(8):
            eo = eop.tile([AC, 4, 4, A], FP8, tag=f"eo{g}", bufs=1)
            nc.sync.dma_start(
                out=eo,
                in_=enco_q[:, g * 4 * 4 * A:(g + 1) * 4 * 4 * A]
                .rearrange("p (b k e) -> p b k e", b=4, k=4),
            )
            eo_tiles.append(eo)

        # fc2 weights on the SAME SP queue AFTER enco: per-queue FIFO gives
        # enco strict priority (the tile scheduler reorders across queues,
        # so cross-engine "late issue" does not work).
        fc2w_sb = const.tile([128, 8, VCP], FP8)
        for n0, nt in _vocab_tiles():
            nc.sync.dma_start(
                out=fc2w_sb[:, :, n0:n0 + nt],
                in_=fc2wq[:].rearrange("p (k j) -> p k j", k=8)[:, :, n0:n0 + nt],
            )

        # tiny consts on gpsimd
        bg_sb = const.tile([128, 12], F32)
        nc.gpsimd.dma_start(out=bg_sb, in_=bg[:].rearrange("(m p) -> p m", p=128))
        smallc = const.tile([BC, 2 * A], F32)
        nc.gpsimd.dma_start(out=smallc, in_=_bc(smallp[:], BC))
        attb_sb = smallc[:, 0:A]
        v64_sb = smallc[:, A:2 * A]
        pg1_sb = const.tile([128, 2], BF16)
        nc.gpsimd.dma_start(out=pg1_sb, in_=pg1b[:].rearrange("(k p) -> p k", p=128))
        pg2_sb = const.tile([AC, 4], FP8)
        nc.gpsimd.dma_start(out=pg2_sb, in_=pg2q[:].rearrange("(k p) -> p k", p=AC))
        pg3_sb = const.tile([128, 4], FP8)
        nc.gpsimd.dma_start(out=pg3_sb, in_=pg3q[:].rearrange("(k p) -> p k", p=128))
        if with_fc1_bias:
            fc1bias_sb = const.tile([BC, 1024], F32)
            nc.gpsimd.dma_start(out=fc1bias_sb, in_=_bc(fc1bias[:], BC))

        attbr_sb = const.tile([1, A], BF16)
        nc.gpsimd.dma_start(out=attbr_sb, in_=attb_bf[:].rearrange("(p a) -> p a", p=1))
        ones_sb = small.tile([1, 1], F32)
        nc.vector.memset(ones_sb, 1.0)
        onesb_sb = small.tile([1, BC], BF16)
        nc.vector.memset(onesb_sb, 1.0)
        c64_sb = small.tile([BC, 1], F32)
        nc.vector.memset(c64_sb, 1.0 / S_ATT)

        # ================= LSTM step (h only) =================
        sg_sb = const.tile([128, 12, BC], F32)  # sig(i), tanh(g), sig(o)
        for m in range(12):
            ps_g = psA.tile([128, BC], F32, tag="mmA")
            for k in range(2):
                nc.tensor.matmul(
                    out=ps_g,
                    lhsT=wih_sb[:, k, m * 128:(m + 1) * 128],
                    rhs=x0T_sb[:, k, :],
                    start=(k == 0), stop=(k == 1),
                )
            func = AF.Tanh if 4 <= m < 8 else AF.Sigmoid
            nc.scalar.activation(
                out=sg_sb[:, m, :], in_=ps_g, func=func,
                bias=bg_sb[:, m:m + 1], scale=1.0,
            )
        cth_sb = const.tile([128, 4, BC], F32)
        nc.vector.tensor_mul(out=cth_sb, in0=sg_sb[:, 0:4, :], in1=sg_sb[:, 4:8, :])
        nc.scalar.activation(out=cth_sb, in_=cth_sb, func=AF.Tanh)
        hT_sb = const.tile([128, 4, BC], BF16)  # h feature-major (attention lhsT)
        nc.vector.tensor_mul(out=hT_sb, in0=sg_sb[:, 8:12, :], in1=cth_sb)

        # dec_inT fp8 [128, 8, 32]: slots 0-3 ctx x64 (100 partitions), 4-7 h x64
        dec_inT = const.tile([128, 8, BC], FP8)
        nc.scalar.activation(out=dec_inT[:, 4:8, :], in_=hT_sb,
                             func=AF.Copy, scale=S_ATT)

        # ================= attention =================
        # e psum accumulates es@Wh + h@Ws + attb (attb via a K=1 ones matmul);
        # tanh bounds e, so softmax needs no max subtraction.
        ps_e = psA.tile([BC, A], F32, tag="mmA")
        for k in range(4):
            nc.tensor.matmul(out=ps_e, lhsT=esT_sb[:, k, :], rhs=whsw_sb[:, k, 0:A],
                             start=(k == 0), stop=False)
        for k in range(4):
            nc.tensor.matmul(out=ps_e, lhsT=hT_sb[:, k, :], rhs=whsw_sb[:, k, A:2 * A],
                             start=False, stop=False)
        nc.tensor.matmul(out=ps_e, lhsT=onesb_sb, rhs=attbr_sb,
                         start=False, stop=True)
        e_sb = const.tile([BC, A], F32)
        nc.scalar.activation(out=e_sb, in_=ps_e, func=AF.Tanh)
        ssum = small.tile([BC, 1], F32)
        nc.scalar.activation(out=e_sb, in_=e_sb, func=AF.Exp, accum_out=ssum)
        rs = small.tile([BC, 1], F32)
        nc.vector.reciprocal(out=rs, in_=ssum)
        att64_sb = const.tile([BC, A], F32)  # att x 64
        nc.vector.scalar_tensor_tensor(out=att64_sb, in0=e_sb, scalar=rs, in1=v64_sb,
                                       op0=ALU.mult, op1=ALU.mult)

        # attT fp8 [100p, 4, 32] via PE transposes (chunks of 100)
        attT_q = const.tile([AC, 4, BC], FP8)
        for c in range(4):
            ps_t = psT.tile([128, BC], F32, tag="tp")
            nc.tensor.transpose(ps_t[:AC, :], att64_sb[:, c * AC:(c + 1) * AC],
                                ident[:BC, :BC])
            nc.scalar.copy(out=attT_q[:, c, :], in_=ps_t[:AC, :])

        # ================= context: ctx64[b,:] = att64[b] @ enco[b] =================
        # Per-row [1,400] psum results; psum->SBUF copies (partition 0 scratch,
        # alternating Scalar/Vector) hide under the enco DMA; 4 chunked
        # SBUF->SBUF DMAs restore batch-major layout.
        ctx_sb = const.tile([BC, A], F32)  # ctx x 64, batch-major
        rows_sb = const.tile([1, BC, A], F32)
        for b in range(BC):
            pst = psC.tile([1, A], F32, tag="psc")
            eo = eo_tiles[b // 4]
            for p in range(2):
                nc.tensor.matmul(
                    out=pst,
                    lhsT=attT_q[:, 2 * p:2 * p + 2, b:b + 1],
                    rhs=eo[:, b % 4, 2 * p:2 * p + 2, :],
                    start=(p == 0), stop=(p == 1),
                    perf_mode=DR,
                )
            if b % 2 == 0:
                nc.scalar.copy(out=rows_sb[:, b, :], in_=pst)
            else:
                nc.vector.tensor_copy(out=rows_sb[:, b, :], in_=pst)
            if b % 8 == 7:
                nc.scalar.dma_start(out=ctx_sb[b - 7:b + 1, :],
                                    in_=rows_sb[:, b - 7:b + 1, :])

        # ctxT fp8 -> dec_inT slots 0-3 (x64 already)
        for c in range(4):
            ps_t = psT.tile([128, BC], F32, tag="tp")
            nc.tensor.transpose(ps_t[:AC, :], ctx_sb[:, c * AC:(c + 1) * AC],
                                ident[:BC, :BC])
            nc.scalar.copy(out=dec_inT[:AC, c, :], in_=ps_t[:AC, :])

        # ================= fc1: z1 = dec_in @ fc1_w^T (+bias) =================
        z1_sb = const.tile([BC, 1024], BF16)
        for nh in range(2):
            ps_z = psA.tile([BC, NT], F32, tag="mmA")
            ns = slice(nh * NT, (nh + 1) * NT)
            for p in range(2):  # ctx pairs, K=100
                nc.tensor.matmul(
                    out=ps_z,
                    lhsT=dec_inT[:AC, 2 * p:2 * p + 2, :],
                    rhs=fc1w_sb[:AC, 2 * p:2 * p + 2, ns],
                    start=(p == 0), stop=False, perf_mode=DR,
                )
            for p in range(2):  # h pairs, K=128
                nc.tensor.matmul(
                    out=ps_z,
                    lhsT=dec_inT[:, 4 + 2 * p:4 + 2 * p + 2, :],
                    rhs=fc1w_sb[:, 4 + 2 * p:4 + 2 * p + 2, ns],
                    start=False, stop=(p == 1), perf_mode=DR,
                )
            if with_fc1_bias:
                nc.vector.scalar_tensor_tensor(
                    out=z1_sb[:, ns], in0=ps_z, scalar=1.0 / (S_ATT * S_FC1W),
                    in1=fc1bias_sb[:, ns], op0=ALU.mult, op1=ALU.add,
                )
            else:
                nc.scalar.activation(out=z1_sb[:, ns], in_=ps_z, func=AF.Copy,
                                     scale=1.0 / (S_ATT * S_FC1W))

        # z1T fp8 (x16) [128, 8, 32] -> DRAM -> AllGather
        z1loc = const.tile([128, 8, BC], FP8)
        for j in range(8):
            ps_t = psT.tile([128, BC], BF16, tag="tp")
            nc.tensor.transpose(ps_t, z1_sb[:, j * 128:(j + 1) * 128],
                                ident_bf[:BC, :BC])
            nc.scalar.activation(out=z1loc[:, j, :], in_=ps_t, func=AF.Copy,
                                 scale=S_Z1)
        nc.scalar.dma_start(out=z1g_c[:], in_=z1loc)
        nc.gpsimd.collective_compute(
            "AllGather", ALU.bypass, replica_groups=RG,
            ins=[z1g_c.opt()], outs=[z1g_full.opt()],
        )

        # ================= gathered z1T, two batch-half tiles =================
        # One DMA pulls all 8 core-chunks in native layout (256B runs), then
        # two engine copies reorg into the k-major z1T layout the DR needs.
        z1cat = const.tile([128, NCORES, 8, BC], FP8)
        nc.scalar.dma_start(
            out=z1cat, in_=z1g_full[:].rearrange("c p f -> p c f"))
        z1T_h0 = const.tile([128, 8, 128], FP8)
        z1T_h1 = const.tile([128, 8, 128], FP8)
        z1T_half = [z1T_h0, z1T_h1]
        for bh, (dst, eng) in enumerate(
                zip(z1T_half, (nc.scalar, nc.vector))):
            src = bass.AP(
                tensor=z1cat.tensor,
                offset=z1cat.offset + bh * 4 * 8 * BC,
                ap=[[NCORES * 8 * BC, 128], [BC, 8], [8 * BC, 4], [1, BC]],
            )
            if bh == 0:
                eng.copy(out=dst, in_=src)
            else:
                eng.tensor_copy(out=dst, in_=src)

        # ================= fc2 + exp + partial denominators =================
        s_acc = small.tile([128, 2], F32, tag="sacc")
        nc.vector.memset(s_acc, 0.0)
        for n0, nt in _vocab_tiles():
            nreal = min(nt, VC - n0)
            for bh in range(2):
                ps_l = psA.tile([128, NT], F32, tag="mmA")
                for p in range(4):
                    nc.tensor.matmul(
                        out=ps_l[:, :nt],
                        lhsT=z1T_half[bh][:, 2 * p:2 * p + 2, :],
                        rhs=fc2w_sb[:, 2 * p:2 * p + 2, n0:n0 + nt],
                        start=(p == 0), stop=(p == 3), perf_mode=DR,
                    )
                o_sb = op_.tile([128, NT], BF16, tag="osb")
                st = small.tile([128, 1], F32, tag="st")
                nc.scalar.activation(
                    out=o_sb[:, :nreal], in_=ps_l[:, :nreal], func=AF.Exp,
                    scale=1.0 / (S_Z1 * S_FC2W), accum_out=st,
                )
                nc.vector.tensor_add(out=s_acc[:, bh:bh + 1],
                                     in0=s_acc[:, bh:bh + 1], in1=st)
                nc.gpsimd.dma_start(
                    out=p_out[bh * 128:(bh + 1) * 128, n0:n0 + nreal],
                    in_=o_sb[:, :nreal],
                )
        nc.scalar.dma_start(out=s_out[:], in_=s_acc)

        # ================= p_gen (off the critical AG path; runs under fc2) ==
        ps_gen = psG.tile([1, BC], F32, tag="gen")
        for k in range(2):  # x0 . pg1  (pg1 x128, bf16)
            nc.tensor.matmul(out=ps_gen, lhsT=pg1_sb[:, k:k + 1], rhs=x0T_sb[:, k, :],
                             start=(k == 0), stop=False)
        for c in range(4):  # ctx . pg2 (ctxT x64 * pg2 x2, fp8)
            nc.tensor.matmul(out=ps_gen, lhsT=pg2_sb[:, c:c + 1],
                             rhs=dec_inT[:AC, c, :], start=False, stop=False)
        for c in range(4):  # h . pg3 (hT x64 * pg3 x2, fp8)
            nc.tensor.matmul(out=ps_gen, lhsT=pg3_sb[:, c:c + 1],
                             rhs=dec_inT[:, 4 + c, :], start=False, stop=(c == 3))
        gen_row = small.tile([1, BC], F32)
        nc.scalar.activation(out=gen_row, in_=ps_gen, func=AF.Sigmoid,
                             scale=1.0 / S_PG)
        nc.scalar.dma_start(out=gen_out[:], in_=gen_row)
        # gen column [32,1] via 1-k matmul with ones
        ps_gc = psT.tile([BC, 1], F32, tag="tp")
        nc.tensor.matmul(out=ps_gc, lhsT=gen_row, rhs=ones_sb)
        gen1m64 = small.tile([BC, 1], F32)
        nc.scalar.activation(out=gen1m64, in_=ps_gc, func=AF.Identity,
                             bias=c64_sb, scale=-1.0 / S_ATT)
        attcopy_sb = const.tile([BC, A], F32)
        nc.vector.tensor_scalar_mul(out=attcopy_sb, in0=att64_sb, scalar1=gen1m64)
        nc.scalar.dma_start(out=attcopy_out[:], in_=attcopy_sb)

    nc.compile()
    return nc


_NC_CACHE = {}


def _get_nc(with_fc1_bias: bool) -> bass.Bass:
    if with_fc1_bias not in _NC_CACHE:
        _NC_CACHE[with_fc1_bias] = build_nc(with_fc1_bias)
    return _NC_CACHE[with_fc1_bias]


RUN_KW = {}        # test.py can set e.g. {"trace": True}
LAST_RESULT = {}   # test.py reads exec_time_ns etc.


def make_in_maps(inputs: dict):
    f32 = lambda a: np.ascontiguousarray(np.asarray(a), dtype=np.float32)
    bf16 = ml_dtypes.bfloat16
    fp8 = ml_dtypes.float8_e4m3  # TRN variant (max +-240)

    def q8(a):
        return np.clip(a, -240.0, 240.0).astype(fp8)

    x = f32(inputs["x"])[:, 0, :]              # [B, I]
    enco = f32(inputs["encoder_outputs"])      # [B, A, A]
    es = f32(inputs["encoder_state"])          # [B, H]
    W_ih = f32(inputs["W_ih"])                 # [4H, I]
    b = f32(inputs["b_ih"]) + f32(inputs["b_hh"])
    Wh_w = f32(inputs["Wh_w"])
    Ws_w = f32(inputs["Ws_w"])
    attb = f32(inputs["Wh_b"]) + f32(inputs["Ws_b"])
    vvec = f32(inputs["v"])
    fc1_w = f32(inputs["fc1_w"])               # [2H, H+A]
    fc1_b = f32(inputs["fc1_b"])
    fc2_w = f32(inputs["fc2_w"])               # [V, 2H]
    pg1 = f32(inputs["pg1_w"])[0]
    pg2 = f32(inputs["pg2_w"])[0]
    pg3 = f32(inputs["pg3_w"])[0]

    with_fc1_bias = bool(np.any(fc1_b != 0.0))

    # i, g, o gate rows (f gate dead: c0 = 0)
    idx = np.r_[0:H, 2 * H:3 * H, 3 * H:4 * H]
    wihT = np.ascontiguousarray(W_ih[idx].T).astype(bf16)   # [I, 1536]
    bgv = np.ascontiguousarray(b[idx])

    whsw = np.concatenate([Wh_w.T, Ws_w.T], axis=1)          # [H, 2A]
    smallp = np.concatenate([attb, vvec * S_ATT])            # [800]
    pg1b = (pg1 * S_PG).astype(bf16)
    pg2qv = q8(pg2 * 2.0)
    pg3qv = q8(pg3 * 2.0)

    # fc1 weights, prearranged in SBUF layout [128, 8, 1024] (x16)
    fc1T = fc1_w.T * S_FC1W                                  # [912, 1024]
    fc1wq = np.zeros((128, 8, 1024), np.float32)
    for k in range(4):
        fc1wq[:AC, k, :] = fc1T[k * AC:(k + 1) * AC]         # ctx chunks of 100
    for k in range(4):
        fc1wq[:, 4 + k, :] = fc1T[A + k * 128:A + (k + 1) * 128]
    fc1wq = q8(fc1wq.reshape(128, 8 * 1024))

    # fc2 weights [128, 8, 6256] (x32), cols >= 6250 zero
    fc2T = fc2_w.T * S_FC2W                                  # [1024, V]
    fc2wq_f = np.zeros((128, 8, VCP), np.float32)

    enco_pk = np.empty((AC, BC * 4 * A), ml_dtypes.float8_e4m3)

    x0T = np.ascontiguousarray(x.T)
    esTm = np.ascontiguousarray(es.T)

    in_maps = []
    enco_q8 = q8(enco)                                       # [B, A, A] fp8
    for c in range(NCORES):
        bs = slice(c * BC, (c + 1) * BC)
        vs = slice(c * VC, (c + 1) * VC)
        fc2wq_f[:, :, :] = 0.0
        for k in range(8):
            fc2wq_f[:, k, :VC] = fc2T[k * 128:(k + 1) * 128, vs]
        fc2wq = q8(fc2wq_f.reshape(128, 8 * VCP))
        # enco chunk-major: [p, (b, k, e)] = enco[bs][b, k*100+p, e]
        ec = enco_q8[bs]                                     # [32, 400, 400]
        epk = ec.reshape(BC, 4, AC, A).transpose(2, 0, 1, 3)  # [100, 32, 4, 400]
        enco_pk = np.ascontiguousarray(epk.reshape(AC, BC * 4 * A))
        m = {
            "x0T": np.ascontiguousarray(x0T[:, bs]).astype(bf16),
            "esT": np.ascontiguousarray(esTm[:, bs]).astype(bf16),
            "wihT": wihT,
            "bg": bgv,
            "whsw": whsw.astype(bf16),
            "smallp": smallp,
            "attb_bf": attb.astype(bf16),
            "pg1b": pg1b,
            "pg2q": pg2qv,
            "pg3q": pg3qv,
            "fc1wq": fc1wq,
            "fc2wq": fc2wq,
            "enco_q": enco_pk,
        }
        if with_fc1_bias:
            m["fc1bias"] = fc1_b
        in_maps.append(m)
    return in_maps, with_fc1_bias


def kernel(**inputs) -> np.ndarray:
    in_maps, with_fc1_bias = make_in_maps(inputs)
    nc = _get_nc(with_fc1_bias)

    res = run_bass_kernel_spmd(nc, in_maps, core_ids=list(range(NCORES)), **RUN_KW)
    results = res.results
    LAST_RESULT["exec_time_ns"] = getattr(res, "exec_time_ns", None)

    oov = int(np.asarray(inputs["max_oov_nums"]))
    ids = np.asarray(inputs["ids"])
    fc2_b = np.asarray(inputs["fc2_b"], dtype=np.float32)

    gen = np.concatenate(
        [np.asarray(results[c]["gen_out"])[0] for c in range(NCORES)])
    p = np.zeros((B, V + oov), dtype=np.float32)
    if np.any(fc2_b != 0.0):
        g = np.exp(fc2_b.astype(np.float64)).astype(np.float32)
        for c in range(NCORES):
            vs = slice(c * VC, (c + 1) * VC)
            p[:, vs] = np.asarray(results[c]["p_out"], dtype=np.float32) * g[None, vs]
        s = p[:, :V].sum(axis=1)
        p[:, :V] *= (gen / s)[:, None]
    else:
        s = np.zeros(B, dtype=np.float32)
        for c in range(NCORES):
            so = np.asarray(results[c]["s_out"])       # [128, 2]
            s += so.T.reshape(B)
        f = (gen / s).astype(np.float32)
        for c in range(NCORES):
            vs = slice(c * VC, (c + 1) * VC)
            p[:, vs] = np.asarray(results[c]["p_out"], dtype=np.float32) * f[:, None]

    att_copy = np.concatenate(
        [np.asarray(results[c]["attcopy_out"]) for c in range(NCORES)], axis=0)
    rows = np.arange(B)[:, None]
    np.add.at(p, (rows, ids), att_copy)
    return p
